# revision 23
# baseline (speedup 1.0000x reference)
"""Trainium2 Bass kernel for nn_AutoregressiveArithmeticTransformer.

6-layer dense transformer: B=16, T=512, E=512, NH=8 heads x HS=64, FF=2048,
V=16, causal attention, pre-LN, learned abacus embedding, logits / 0.8.

Strategy: data-parallel over batch across 8 NeuronCores (2 sequences per
core, no collectives). Activations live feature-major in SBUF
([E-partitions, tokens]); the residual stream is bf16; weights are streamed
per-layer in bf16; all matmuls run in bf16 with fp32 PSUM accumulation.

v2 over the original baseline:
  - layer-0 LN1 + Q/K/V projections precomputed on the host (the embedding
    gather is already host-side); q0/k0/v0 are DMA'd directly.
  - softmax denominator ones-matmuls col-tiled (M=64 pairs at tile
    positions (0,0)/(0,64)) so the two heads of a pair run concurrently
    in the PE array; same for the attention-V matmuls (as before).
  - score pairs land in one 2-bank PSUM tile so exp is ONE activation op
    per key-chunk ([128, 2, njw]); reciprocal and the o*(1/den) multiply
    are one [128,512] op per (seq, head-pair).
  - q/k PSUM results share a 2-bank tile -> single cast per (hp, tt);
    V-projection copies merged in pairs; FFN1 relu merged in pairs.
  - bf16 residual stream: no fp32->bf16 cast before LN stats matmuls.
  - LN apply multiplies and causal tri-mask multiplies run on the
    otherwise-idle GpSimd engine.
"""

import numpy as np
import ml_dtypes

import concourse.bacc as bacc
import concourse.tile as tile
from concourse import mybir

F32 = mybir.dt.float32
BF16 = mybir.dt.bfloat16
AF = mybir.ActivationFunctionType
OP = mybir.AluOpType

# Model constants (hardcoded per contest contract)
V, E, NH, HS, FF, NB, L = 16, 512, 8, 64, 2048, 6, 512
B, T = 16, 512
TEMP = 1.0 * 0.8
EPS = 1e-5
SCALE = HS ** -0.5  # 0.125

NCORES = 8
SEQ = 2              # sequences per core
NTOK = SEQ * T       # 1024 tokens per core
C = E // 128         # 4 E-chunks
CF = FF // 128       # 16 FF-chunks
HP = NH // 2         # 4 head-pairs
NJ = T // 128        # 4 tk chunks per sequence

_PROGRAM_CACHE = {}


def _emit_ln_tt(nc, pools, h_t, ones_t, eps2_t, g_ap, b_ap, trivial, tt, xn):
    """One token-tile of LayerNorm from bf16 h_t into caller-alloc'd xn.

    Chain is kept DVE-local with a single ACT hop (Sqrt):
      mu = s1/E; var = s2/E - mu^2; sig = sqrt(var + eps); r = 1/sig;
      xn = h*r - mu*r.
    """
    stats, stats_bf = pools["stats"], pools["stats_bf"]
    ps1 = pools["ps1"]
    sq = pools["sq"]
    sl = slice(tt * 512, tt * 512 + 512)
    s1 = ps1.tile([128, 512], F32, tag="ps1")
    s2 = ps1.tile([128, 512], F32, tag="ps1")
    for c in range(C):
        nc.vector.tensor_tensor(sq[:, c, sl], h_t[:, c, sl], h_t[:, c, sl],
                                OP.mult)
        nc.tensor.matmul(s1[:], ones_t[:], h_t[:, c, sl],
                         start=(c == 0), stop=(c == C - 1))
        nc.tensor.matmul(s2[:], ones_t[:], sq[:, c, sl],
                         start=(c == 0), stop=(c == C - 1))
    mu = stats.tile([128, 512], F32, tag="stats")
    nc.vector.tensor_scalar(out=mu[:], in0=s1[:],
                            scalar1=1.0 / float(E), scalar2=None, op0=OP.mult)
    msq = stats.tile([128, 512], F32, tag="stats")
    nc.vector.tensor_tensor(msq[:], mu[:], mu[:], OP.mult)
    var = stats.tile([128, 512], F32, tag="stats")
    nc.vector.scalar_tensor_tensor(out=var[:], in0=s2[:],
                                   scalar=1.0 / float(E), in1=msq[:],
                                   op0=OP.mult, op1=OP.subtract)
    std = stats.tile([128, 512], F32, tag="stats")
    nc.scalar.activation(std[:], var[:], AF.Sqrt, bias=eps2_t[:])
    rc = stats.tile([128, 512], F32, tag="stats")
    nc.vector.reciprocal_approx_fast(out=rc[:], in_=std[:])
    r_bf = stats_bf.tile([128, 512], BF16, tag="r_bf")
    nc.scalar.copy(r_bf[:], rc[:])
    z_bf = stats_bf.tile([128, 512], BF16, tag="z_bf")
    nc.vector.tensor_tensor(z_bf[:], mu[:], rc[:], OP.mult)
    if xn is not None:
        for c in range(C):
            nc.vector.tensor_tensor(xn[:, c, sl], h_t[:, c, sl], r_bf[:],
                                    OP.mult)
            nc.vector.tensor_tensor(xn[:, c, sl], xn[:, c, sl], z_bf[:],
                                    OP.subtract)
            if not trivial:
                nc.vector.tensor_scalar(out=xn[:, c, sl], in0=xn[:, c, sl],
                                        scalar1=g_ap[:, c:c + 1],
                                        scalar2=b_ap[:, c:c + 1],
                                        op0=OP.mult, op1=OP.add)
    return r_bf, z_bf


def _alloc_xn(pools):
    return pools["scr"].tile([128, C, NTOK], BF16, tag="scratch", name="xnt")


def _emit_ln(nc, pools, h_t, ones_t, eps2_t, g_ap, b_ap, trivial):
    xn = _alloc_xn(pools)
    rz = []
    for tt in range(2):
        rz.append(_emit_ln_tt(nc, pools, h_t, ones_t, eps2_t, g_ap, b_ap,
                              trivial, tt, xn))
    return xn, rz


def build_program(ln_trivial, bias_zero, nb_run=NB, ln_general_params=True):
    """Build the Bass program.

    ln_trivial: list of NB*2+1 bools (ln1/ln2 per layer then lnf); when True
    the g/b application op is skipped.  bias_zero: (pb, fb1, fb2) all-zero
    flags enabling merged residual/relu fast paths."""
    pbz, fb1z, fb2z = bias_zero
    nc = bacc.Bacc(None, target_bir_lowering=False)

    h0_d = nc.dram_tensor("h0", [128, C * NTOK], BF16, kind="ExternalInput")
    qk0_d = nc.dram_tensor("qk0", [128, HP * 2 * NTOK], BF16,
                           kind="ExternalInput")
    v0_d = nc.dram_tensor("v0", [128, SEQ * NJ * 512], BF16,
                          kind="ExternalInput")
    wq_d = nc.dram_tensor("wq", [NB, 128, C * 512], BF16, kind="ExternalInput")
    wk_d = nc.dram_tensor("wk", [NB, 128, C * 512], BF16, kind="ExternalInput")
    wv_d = nc.dram_tensor("wv", [NB, 128, C * 512], BF16, kind="ExternalInput")
    pw_d = nc.dram_tensor("pw", [NB, 128, C * 512], BF16, kind="ExternalInput")
    f1_d = nc.dram_tensor("f1", [NB, 128, C * FF], BF16, kind="ExternalInput")
    f2_d = nc.dram_tensor("f2", [NB, 128, CF * 512], BF16, kind="ExternalInput")
    pb_d = nc.dram_tensor("pb", [128, NB * C], F32, kind="ExternalInput")
    fb1_d = nc.dram_tensor("fb1", [128, NB * CF], F32, kind="ExternalInput")
    fb2_d = nc.dram_tensor("fb2", [128, NB * C], F32, kind="ExternalInput")
    ow_d = nc.dram_tensor("ow", [128, C * V], BF16, kind="ExternalInput")
    ob_d = nc.dram_tensor("ob", [V, 1], F32, kind="ExternalInput")
    tri_d = nc.dram_tensor("tri", [128, 128], BF16, kind="ExternalInput")
    # negated column sums for LN-corrected first blocks
    csqk_d = nc.dram_tensor("csqk", [128, NB * 2], F32, kind="ExternalInput")
    csf1_d = nc.dram_tensor("csf1", [128, NB * 4], F32, kind="ExternalInput")
    csow_d = nc.dram_tensor("csow", [V, 1], F32, kind="ExternalInput")
    lng_d = lnb_d = None
    if ln_general_params:
        lng_d = nc.dram_tensor("lng", [128, (2 * NB + 1) * C], F32,
                               kind="ExternalInput")
        lnb_d = nc.dram_tensor("lnb", [128, (2 * NB + 1) * C], F32,
                               kind="ExternalInput")
    out_d = nc.dram_tensor("logits", [V, NTOK], F32, kind="ExternalOutput")

    from contextlib import ExitStack
    with ExitStack() as ctx:
        tc = ctx.enter_context(tile.TileContext(nc))
        consts = ctx.enter_context(tc.tile_pool(name="consts", bufs=1))
        hpool = ctx.enter_context(tc.tile_pool(name="hpool", bufs=1))
        wqkv = ctx.enter_context(tc.tile_pool(name="wqkv", bufs=1))
        wff1 = ctx.enter_context(tc.tile_pool(name="wff1", bufs=1))
        wff2 = ctx.enter_context(tc.tile_pool(name="wff2", bufs=1))
        scr = ctx.enter_context(tc.tile_pool(name="scr", bufs=3))
        sqpool = ctx.enter_context(tc.tile_pool(name="sqp", bufs=1))
        qk = ctx.enter_context(tc.tile_pool(name="qk", bufs=2))
        vt = ctx.enter_context(tc.tile_pool(name="vt", bufs=1))
        pp = ctx.enter_context(tc.tile_pool(name="pp", bufs=3))
        osb = ctx.enter_context(tc.tile_pool(name="osb", bufs=1))
        ffa = ctx.enter_context(tc.tile_pool(name="ffa", bufs=2))
        corr = ctx.enter_context(tc.tile_pool(name="corr", bufs=2))
        stats = ctx.enter_context(tc.tile_pool(name="stats", bufs=5))
        rdp = ctx.enter_context(tc.tile_pool(name="rdp", bufs=2))
        stats_bf = ctx.enter_context(tc.tile_pool(name="stats_bf", bufs=3))
        ps2 = ctx.enter_context(tc.tile_pool(name="ps2", bufs=3, space="PSUM"))
        ps1 = ctx.enter_context(tc.tile_pool(name="ps1", bufs=2, space="PSUM"))

        sq_t = sqpool.tile([128, C, NTOK], BF16)
        pools = {"scr": scr, "sq": sq_t, "stats": stats, "rdp": rdp,
                 "stats_bf": stats_bf, "ps2": ps2, "ps1": ps1}

        ones_t = consts.tile([128, 128], BF16)
        nc.gpsimd.memset(ones_t[:], 1.0)
        eps2_t = consts.tile([128, 1], F32)
        nc.gpsimd.memset(eps2_t[:], EPS)
        tri_t = consts.tile([128, 128], BF16)
        nc.sync.dma_start(tri_t[:], tri_d[:])
        pb_t = consts.tile([128, NB * C], F32)
        nc.sync.dma_start(pb_t[:], pb_d[:])
        fb1_t = consts.tile([128, NB * CF], F32)
        nc.sync.dma_start(fb1_t[:], fb1_d[:])
        fb2_t = consts.tile([128, NB * C], F32)
        nc.sync.dma_start(fb2_t[:], fb2_d[:])
        ow_t = consts.tile([128, C, V], BF16)
        nc.sync.dma_start(ow_t[:], ow_d[:].rearrange("p (c v) -> p c v", v=V))
        ob_t = consts.tile([V, 1], F32)
        nc.sync.dma_start(ob_t[:], ob_d[:])
        csqk_t = consts.tile([128, NB, 2], F32)
        nc.sync.dma_start(csqk_t[:], csqk_d[:].rearrange(
            "p (l u) -> p l u", u=2))
        csf1_t = consts.tile([128, NB, 4], F32)
        nc.sync.dma_start(csf1_t[:], csf1_d[:].rearrange(
            "p (l u) -> p l u", u=4))
        csow_t = consts.tile([V, 1], F32)
        nc.sync.dma_start(csow_t[:], csow_d[:])
        lng_t = lnb_t = None
        if ln_general_params:
            lng_t = consts.tile([128, 2 * NB + 1, C], F32)
            nc.sync.dma_start(lng_t[:], lng_d[:].rearrange(
                "p (l c) -> p l c", c=C))
            lnb_t = consts.tile([128, 2 * NB + 1, C], F32)
            nc.sync.dma_start(lnb_t[:], lnb_d[:].rearrange(
                "p (l c) -> p l c", c=C))

        h_t = hpool.tile([128, C, NTOK], BF16)
        nc.sync.dma_start(h_t[:], h0_d[:].rearrange(
            "p (c t) -> p c t", t=NTOK))

        def ln_params(idx):
            if ln_general_params and not ln_trivial[idx]:
                return lng_t[:, idx, :], lnb_t[:, idx, :], False
            return None, None, True

        for i in range(nb_run):
            # ---- load this layer's weights ----
            if i > 0:
                wq_t = wqkv.tile([128, C, 512], BF16, tag="wq")
                nc.sync.dma_start(wq_t[:], wq_d[i].rearrange(
                    "p (c m) -> p c m", m=512))
                wk_t = wqkv.tile([128, C, 512], BF16, tag="wk")
                nc.sync.dma_start(wk_t[:], wk_d[i].rearrange(
                    "p (c m) -> p c m", m=512))
                wv_t = wqkv.tile([128, C, 512], BF16, tag="wv")
                nc.sync.dma_start(wv_t[:], wv_d[i].rearrange(
                    "p (c m) -> p c m", m=512))
            pw_t = wqkv.tile([128, C, 512], BF16, tag="pw")
            nc.sync.dma_start(pw_t[:], pw_d[i].rearrange(
                "p (c m) -> p c m", m=512))
            f1_t = wff1.tile([128, C, FF], BF16, tag="f1")
            nc.sync.dma_start(f1_t[:], f1_d[i].rearrange(
                "p (c m) -> p c m", m=FF))
            f2_t = wff2.tile([128, CF, 512], BF16, tag="f2")
            nc.sync.dma_start(f2_t[:], f2_d[i].rearrange(
                "p (c m) -> p c m", m=512))

            # ---- LN1 output for this layer (layer 0: host-computed;
            #      others peeled into the previous layer's FFN emission) ----
            xn = xn_next if i > 0 else None
            qk_corr = i > 0 and ln_trivial[2 * i]

            # ---- corrected Q/K for head-pair 0: matmuls run on h_t so the
            #      PE has work while the LN1 chain computes r/z; the drain
            #      applies q = (Wq^T h)*r - z*colsum(Wq). ----
            qk_hp0 = None
            if qk_corr:
                qk_hp0 = qk.tile([128, 2, NTOK], BF16, tag="qk")
                msl0 = slice(0, 128)
                for tt in range(2):
                    r1, z1 = rz1_next[tt]
                    sl = slice(tt * 512, tt * 512 + 512)
                    qkp = ps2.tile([128, 2, 512], F32, tag="ps2")
                    for c in range(C):
                        nc.tensor.matmul(qkp[:, 0, :], wq_t[:, c, msl0],
                                         h_t[:, c, sl],
                                         start=(c == 0), stop=(c == C - 1))
                        nc.tensor.matmul(qkp[:, 1, :], wk_t[:, c, msl0],
                                         h_t[:, c, sl],
                                         start=(c == 0), stop=(c == C - 1))
                    nc.vector.tensor_tensor(
                        qk_hp0[:, :, sl], qkp[:],
                        r1[:, None, :].to_broadcast((128, 2, 512)),
                        OP.mult)
                    for u in range(2):
                        nc.vector.scalar_tensor_tensor(
                            out=qk_hp0[:, u, sl], in0=z1[:],
                            scalar=csqk_t[:, i, u:u + 1],
                            in1=qk_hp0[:, u, sl], op0=OP.mult, op1=OP.add)

            # ---- V, token-major: vt[tk, hd*64+d] ----
            vt_t = vt.tile([128, SEQ * NJ, 512], BF16, tag="vt")
            if i == 0:
                nc.sync.dma_start(vt_t[:], v0_d[:].rearrange(
                    "p (g m) -> p g m", m=512))
            else:
                for jp in range(SEQ * NJ // 2):
                    vp = ps2.tile([128, 2, 512], F32, tag="ps2")
                    for u in range(2):
                        jg = jp * 2 + u
                        for c in range(C):
                            nc.tensor.matmul(
                                vp[:, u, :],
                                xn[:, c, jg * 128:(jg + 1) * 128],
                                wv_t[:, c, :],
                                start=(c == 0), stop=(c == C - 1))
                    nc.scalar.copy(vt_t[:, jp * 2:jp * 2 + 2, :], vp[:])

            o_t = osb.tile([128, C, NTOK], BF16, tag="o")

            def emit_den_o(s, hp, p_t):
                base = s * T
                den = ps1.tile([128, 512], F32, tag="ps1")
                for j in range(NJ):
                    off = j * 128
                    njw = T - off
                    for h2 in range(2):
                        nc.tensor.matmul(den[h2 * 64:h2 * 64 + 64, off:512],
                                         ones_t[:, 0:64],
                                         p_t[:, h2, j, 0:njw],
                                         start=(j == 0), stop=(j == NJ - 1),
                                         skip_group_check=True)
                rd = rdp.tile([128, 512], F32, tag="rd")
                nc.vector.reciprocal_approx_fast(out=rd[:], in_=den[:])
                op_ps = ps1.tile([128, 512], F32, tag="ps1")
                for j in range(NJ):
                    off = j * 128
                    njw = T - off
                    for h2 in range(2):
                        head = hp * 2 + h2
                        nc.tensor.matmul(
                            op_ps[h2 * 64:h2 * 64 + 64, off:T],
                            vt_t[:, s * NJ + j, head * 64:head * 64 + 64],
                            p_t[:, h2, j, 0:njw],
                            start=(j == 0), stop=(j == NJ - 1),
                            skip_group_check=True)
                nc.vector.tensor_tensor(o_t[:, hp, base:base + T],
                                        op_ps[:, 0:T], rd[:], OP.mult)

            pending = None
            for hp in range(HP):
                msl = slice(hp * 128, (hp + 1) * 128)
                if hp == 0 and qk_hp0 is not None:
                    qk_t = qk_hp0
                else:
                    qk_t = qk.tile([128, 2, NTOK], BF16, tag="qk")
                    if i == 0:
                        nc.sync.dma_start(
                            qk_t[:],
                            qk0_d[:, hp * 2 * NTOK:(hp + 1) * 2 * NTOK]
                            .rearrange("p (q t) -> p q t", t=NTOK))
                    else:
                        for tt in range(2):
                            sl = slice(tt * 512, tt * 512 + 512)
                            qkp = ps2.tile([128, 2, 512], F32, tag="ps2")
                            for c in range(C):
                                nc.tensor.matmul(qkp[:, 0, :],
                                                 wq_t[:, c, msl],
                                                 xn[:, c, sl],
                                                 start=(c == 0),
                                                 stop=(c == C - 1))
                                nc.tensor.matmul(qkp[:, 1, :],
                                                 wk_t[:, c, msl],
                                                 xn[:, c, sl],
                                                 start=(c == 0),
                                                 stop=(c == C - 1))
                            nc.scalar.copy(qk_t[:, :, sl], qkp[:])

                for s in range(SEQ):
                    base = s * T
                    p_t = pp.tile([128, 2, NJ, 512], BF16, tag="p")
                    for j in range(NJ):
                        off = j * 128
                        njw = T - off
                        sT = ps2.tile([128, 2, 512], F32, tag="ps2")
                        for h2 in range(2):
                            dsl = slice(h2 * 64, h2 * 64 + 64)
                            nc.tensor.matmul(
                                sT[:, h2, 0:njw],
                                qk_t[dsl, 1, base + off:base + off + 128],
                                qk_t[dsl, 0, base + off:base + T],
                                start=True, stop=True)
                        nc.scalar.activation(
                            p_t[:, :, j, 0:njw], sT[:, :, 0:njw],
                            AF.Exp, scale=SCALE)
                        nc.vector.tensor_tensor(
                            p_t[:, :, j, 0:128], p_t[:, :, j, 0:128],
                            tri_t[:, None, :].to_broadcast(
                                (128, 2, 128)), OP.mult)
                    if pending is not None:
                        emit_den_o(*pending)
                    pending = (s, hp, p_t)
            emit_den_o(*pending)

            # ---- attention out projection + residual ----
            for tt in range(2):
                sl = slice(tt * 512, tt * 512 + 512)
                for mcp in range(C // 2):
                    pj = ps2.tile([128, 2, 512], F32, tag="ps2")
                    for u in range(2):
                        mc = mcp * 2 + u
                        for c in range(C):
                            nc.tensor.matmul(
                                pj[:, u, :],
                                pw_t[:, c, mc * 128:(mc + 1) * 128],
                                o_t[:, c, sl],
                                start=(c == 0), stop=(c == C - 1))
                    if pbz:
                        nc.vector.tensor_tensor(
                            h_t[:, mcp * 2:mcp * 2 + 2, sl], pj[:],
                            h_t[:, mcp * 2:mcp * 2 + 2, sl], OP.add)
                    else:
                        for u in range(2):
                            mc = mcp * 2 + u
                            nc.vector.scalar_tensor_tensor(
                                out=h_t[:, mc, sl], in0=pj[:, u, :],
                                scalar=pb_t[:, i * C + mc:i * C + mc + 1],
                                in1=h_t[:, mc, sl], op0=OP.add, op1=OP.add)

            # ---- LN2 + FFN (token-tile split) ----
            g_ap, b_ap, triv = ln_params(2 * i + 1)
            ffn_corr = triv
            xn2 = _alloc_xn(pools)
            r2_0, z2_0 = _emit_ln_tt(nc, pools, h_t, ones_t, eps2_t,
                                     g_ap, b_ap, triv, 0, xn2)
            # corrected first FFN1 blocks (tt=0): matmuls on h_t fill the PE
            # while the LN2 chain runs; drain applies r/z + colsum correction.
            corr_fa = []
            if ffn_corr:
                for mfp in range(2):
                    fp = ps2.tile([128, 2, 512], F32, tag="ps2")
                    for u in range(2):
                        mf = mfp * 2 + u
                        for c in range(C):
                            nc.tensor.matmul(
                                fp[:, u, :],
                                f1_t[:, c, mf * 128:(mf + 1) * 128],
                                h_t[:, c, 0:512],
                                start=(c == 0), stop=(c == C - 1))
                    wtmp = corr.tile([128, 2, 512], F32, tag="corr")
                    nc.vector.tensor_tensor(
                        wtmp[:], fp[:],
                        r2_0[:, None, :].to_broadcast((128, 2, 512)), OP.mult)
                    for u in range(2):
                        mf = mfp * 2 + u
                        nc.vector.scalar_tensor_tensor(
                            out=wtmp[:, u, :], in0=z2_0[:],
                            scalar=csf1_t[:, i, mf:mf + 1],
                            in1=wtmp[:, u, :], op0=OP.mult, op1=OP.add)
                    corr_fa.append(wtmp)
            _emit_ln_tt(nc, pools, h_t, ones_t, eps2_t, g_ap, b_ap, triv,
                        1, xn2)

            for tt in range(2):
                sl = slice(tt * 512, tt * 512 + 512)
                fa = ffa.tile([128, CF, 512], BF16, tag="fa")
                for mfp in range(CF // 2):
                    if tt == 0 and ffn_corr and mfp < 2:
                        src = corr_fa[mfp][:]
                    else:
                        fp = ps2.tile([128, 2, 512], F32, tag="ps2")
                        for u in range(2):
                            mf = mfp * 2 + u
                            for c in range(C):
                                nc.tensor.matmul(
                                    fp[:, u, :],
                                    f1_t[:, c, mf * 128:(mf + 1) * 128],
                                    xn2[:, c, sl],
                                    start=(c == 0), stop=(c == C - 1))
                        src = fp[:]
                    if fb1z:
                        nc.scalar.activation(
                            fa[:, mfp * 2:mfp * 2 + 2, :], src, AF.Relu)
                    else:
                        for u in range(2):
                            mf = mfp * 2 + u
                            nc.scalar.activation(
                                fa[:, mf, :], src[:, u, :], AF.Relu,
                                bias=fb1_t[:, i * CF + mf:i * CF + mf + 1])
                for mcp in range(C // 2):
                    f2p = ps2.tile([128, 2, 512], F32, tag="ps2")
                    for u in range(2):
                        for c16 in range(CF):
                            nc.tensor.matmul(
                                f2p[:, u, :],
                                f2_t[:, c16,
                                     (mcp * 2 + u) * 128:
                                     (mcp * 2 + u + 1) * 128],
                                fa[:, c16, :],
                                start=(c16 == 0), stop=(c16 == CF - 1))
                    if fb2z:
                        nc.vector.tensor_tensor(
                            h_t[:, mcp * 2:mcp * 2 + 2, sl], f2p[:],
                            h_t[:, mcp * 2:mcp * 2 + 2, sl], OP.add)
                    else:
                        for u in range(2):
                            mc = mcp * 2 + u
                            nc.vector.scalar_tensor_tensor(
                                out=h_t[:, mc, sl], in0=f2p[:, u, :],
                                scalar=fb2_t[:, i * C + mc:i * C + mc + 1],
                                in1=h_t[:, mc, sl], op0=OP.add, op1=OP.add)
                # peel next layer's LN1(tt) here so its scalar/vector chain
                # hides behind the other token-tile's FFN matmuls
                if i + 1 < nb_run:
                    if tt == 0:
                        xn_next = _alloc_xn(pools)
                        rz1_next = []
                    g_ap, b_ap, triv = ln_params(2 * (i + 1))
                    rz1_next.append(_emit_ln_tt(nc, pools, h_t, ones_t,
                                                eps2_t, g_ap, b_ap, triv,
                                                tt, xn_next))

        # ---- final LN + logits (corrected: logits matmuls run on h_t) ----
        g_ap, b_ap, triv = (ln_params(2 * NB) if nb_run == NB
                            else (None, None, True))
        lg_sb = consts.tile([V, NTOK], F32)
        if triv:
            rzf = [_emit_ln_tt(nc, pools, h_t, ones_t, eps2_t, g_ap, b_ap,
                               triv, tt, None) for tt in range(2)]
            for tt in range(2):
                sl = slice(tt * 512, tt * 512 + 512)
                lg = ps1.tile([V, 512], F32, tag="ps1")
                for c in range(C):
                    nc.tensor.matmul(lg[:], ow_t[:, c, :], h_t[:, c, sl],
                                     start=(c == 0), stop=(c == C - 1))
                rf, zf = rzf[tt]
                w16 = rdp.tile([V, 512], F32, tag="w16")
                nc.vector.tensor_tensor(w16[:], lg[:], rf[0:V, :], OP.mult)
                nc.vector.scalar_tensor_tensor(
                    out=w16[:], in0=zf[0:V, :], scalar=csow_t[:],
                    in1=w16[:], op0=OP.mult, op1=OP.add)
                nc.vector.tensor_scalar_add(lg_sb[:, sl], w16[:], ob_t[:])
        else:
            xnf, _ = _emit_ln(nc, pools, h_t, ones_t, eps2_t, g_ap, b_ap,
                              triv)
            for tt in range(2):
                sl = slice(tt * 512, tt * 512 + 512)
                lg = ps1.tile([V, 512], F32, tag="ps1")
                for c in range(C):
                    nc.tensor.matmul(lg[:], ow_t[:, c, :], xnf[:, c, sl],
                                     start=(c == 0), stop=(c == C - 1))
                nc.vector.tensor_scalar_add(lg_sb[:, sl], lg[:], ob_t[:])
        nc.sync.dma_start(out_d[:], lg_sb[:])

    nc.finalize()
    return nc


def prepare_inputs(inputs):
    """Host-side preprocessing: embedding gather, layer-0 LN1+QKV, weight
    layout + bf16 cast.  Returns (shared_map, per_core_maps, flags)."""
    f32 = np.float32
    bf16 = ml_dtypes.bfloat16
    x = np.asarray(inputs["x"]).astype(np.int64)
    emb = np.asarray(inputs["emb"], dtype=f32)
    pos = np.asarray(inputs["pos"], dtype=f32)

    positions = np.minimum(np.arange(T), L - 1)
    h0 = (emb[x] + pos[positions][None, :, :]).astype(bf16).astype(f32)

    # layer-0 LN1 + Q/K/V on host (fp32, then bf16)
    g1 = np.asarray(inputs["ln1_g"][0], dtype=f32)
    b1 = np.asarray(inputs["ln1_b"][0], dtype=f32)
    mu = h0.mean(-1, keepdims=True)
    var = np.square(h0 - mu).mean(-1, keepdims=True)
    xn0 = ((h0 - mu) / np.sqrt(var + EPS) * g1 + b1).astype(bf16).astype(f32)
    wq0 = np.asarray(inputs["wq"][0], dtype=f32).astype(bf16).astype(f32)
    wk0 = np.asarray(inputs["wk"][0], dtype=f32).astype(bf16).astype(f32)
    wv0 = np.asarray(inputs["wv"][0], dtype=f32).astype(bf16).astype(f32)
    # [B, T, NH*HS] with head-major feature order
    q0 = np.einsum('bte,hed->bthd', xn0, wq0).reshape(B, T, NH * HS)
    k0 = np.einsum('bte,hed->bthd', xn0, wk0).reshape(B, T, NH * HS)
    v0 = np.einsum('bte,hed->bthd', xn0, wv0).reshape(B, T, NH * HS)

    def to_dev_lhst(mat, kchunks, mcols):
        m = np.ascontiguousarray(mat.astype(bf16))
        return m.reshape(kchunks, 128, mcols).transpose(1, 0, 2).reshape(
            128, kchunks * mcols)

    wq = np.asarray(inputs["wq"], dtype=f32)
    wk = np.asarray(inputs["wk"], dtype=f32)
    wv = np.asarray(inputs["wv"], dtype=f32)
    pw = np.asarray(inputs["proj_w"], dtype=f32)
    f1 = np.asarray(inputs["ff_w1"], dtype=f32)
    f2 = np.asarray(inputs["ff_w2"], dtype=f32)

    wq_dev = np.stack([to_dev_lhst(wq[i].transpose(1, 0, 2).reshape(E, NH * HS),
                                   C, 512) for i in range(NB)])
    wk_dev = np.stack([to_dev_lhst(wk[i].transpose(1, 0, 2).reshape(E, NH * HS),
                                   C, 512) for i in range(NB)])
    wv_dev = np.stack([to_dev_lhst(wv[i].transpose(1, 0, 2).reshape(E, NH * HS),
                                   C, 512) for i in range(NB)])
    pw_dev = np.stack([to_dev_lhst(pw[i], C, 512) for i in range(NB)])
    f1_dev = np.stack([to_dev_lhst(f1[i], C, FF) for i in range(NB)])
    f2_dev = np.stack([to_dev_lhst(f2[i], CF, 512) for i in range(NB)])

    def vec_dev(v, chunks):
        return np.ascontiguousarray(v.astype(f32).reshape(chunks, 128).T)

    pb = np.asarray(inputs["proj_b"], dtype=f32)
    fb1 = np.asarray(inputs["ff_b1"], dtype=f32)
    fb2 = np.asarray(inputs["ff_b2"], dtype=f32)
    bias_zero = (bool(np.all(pb == 0.0)), bool(np.all(fb1 == 0.0)),
                 bool(np.all(fb2 == 0.0)))
    pb_dev = np.concatenate([vec_dev(pb[i], C) for i in range(NB)], axis=1)
    fb1_dev = np.concatenate([vec_dev(fb1[i], CF) for i in range(NB)], axis=1)
    fb2_dev = np.concatenate([vec_dev(fb2[i], C) for i in range(NB)], axis=1)
    ow_dev = to_dev_lhst(np.asarray(inputs["out_w"], dtype=f32) / TEMP, C, V)
    ob_dev = (np.asarray(inputs["out_b"], dtype=f32) / TEMP).reshape(V, 1)
    tri_dev = np.triu(np.ones((128, 128), dtype=f32)).astype(bf16)

    # negated column sums (of the bf16-cast weights) for corrected blocks
    def neg_cs(mat, cols):
        mb = mat.astype(bf16).astype(f32)
        return -mb[:, cols].sum(axis=0)

    csqk_dev = np.zeros((128, NB * 2), f32)
    csf1_dev = np.zeros((128, NB * 4), f32)
    for i in range(NB):
        wq_flat = wq[i].transpose(1, 0, 2).reshape(E, NH * HS)
        wk_flat = wk[i].transpose(1, 0, 2).reshape(E, NH * HS)
        csqk_dev[:, i * 2 + 0] = neg_cs(wq_flat, slice(0, 128))
        csqk_dev[:, i * 2 + 1] = neg_cs(wk_flat, slice(0, 128))
        for mf in range(4):
            csf1_dev[:, i * 4 + mf] = neg_cs(
                f1[i], slice(mf * 128, (mf + 1) * 128))
    csow_dev = np.ascontiguousarray(
        neg_cs(np.asarray(inputs["out_w"], dtype=f32) / TEMP,
               slice(0, V)).reshape(V, 1))

    gs, bs, ln_trivial = [], [], []
    for i in range(NB):
        for nm_g, nm_b in (("ln1_g", "ln1_b"), ("ln2_g", "ln2_b")):
            g = np.asarray(inputs[nm_g][i], dtype=f32)
            b = np.asarray(inputs[nm_b][i], dtype=f32)
            gs.append(vec_dev(g, C))
            bs.append(vec_dev(b, C))
            ln_trivial.append(bool(np.all(g == 1.0) and np.all(b == 0.0)))
    g = np.asarray(inputs["lnf_g"], dtype=f32)
    b = np.asarray(inputs["lnf_b"], dtype=f32)
    gs.append(vec_dev(g, C))
    bs.append(vec_dev(b, C))
    ln_trivial.append(bool(np.all(g == 1.0) and np.all(b == 0.0)))
    lng_dev = np.concatenate(gs, axis=1)
    lnb_dev = np.concatenate(bs, axis=1)

    shared = {
        "wq": wq_dev, "wk": wk_dev, "wv": wv_dev, "pw": pw_dev,
        "f1": f1_dev, "f2": f2_dev, "pb": pb_dev, "fb1": fb1_dev,
        "fb2": fb2_dev, "ow": ow_dev, "ob": ob_dev, "tri": tri_dev,
        "lng": lng_dev, "lnb": lnb_dev, "csqk": csqk_dev, "csf1": csf1_dev,
        "csow": csow_dev,
    }

    per_core = []
    for core in range(NCORES):
        csl = slice(SEQ * core, SEQ * core + SEQ)

        def featmaj(a):                      # [SEQ, T, F] -> [128, F/128*NTOK]
            fT = a[csl].transpose(2, 0, 1).reshape(-1, NTOK)   # [F, NTOK]
            ch = fT.shape[0] // 128
            return np.ascontiguousarray(
                fT.reshape(ch, 128, NTOK).transpose(1, 0, 2).reshape(
                    128, ch * NTOK).astype(bf16))

        h0c = featmaj(h0)                       # [128, C*NTOK]
        # qk0: [128, hp, {q,k}, NTOK]; partition = h2*64+d of the pair
        qf = q0[csl].transpose(2, 0, 1).reshape(NH * HS, NTOK)  # [512, NTOK]
        kf = k0[csl].transpose(2, 0, 1).reshape(NH * HS, NTOK)
        qk0c = np.empty((128, HP, 2, NTOK), dtype=f32)
        for hp in range(HP):
            qk0c[:, hp, 0] = qf[hp * 128:(hp + 1) * 128]
            qk0c[:, hp, 1] = kf[hp * 128:(hp + 1) * 128]
        qk0c = np.ascontiguousarray(
            qk0c.reshape(128, HP * 2 * NTOK).astype(bf16))
        # v0: token-major [128, SEQ*NJ, 512]
        vtok = v0[csl].reshape(NTOK, NH * HS)          # [NTOK, 512]
        v0c = np.ascontiguousarray(
            vtok.reshape(SEQ * NJ, 128, NH * HS).transpose(1, 0, 2).reshape(
                128, SEQ * NJ * 512).astype(bf16))
        per_core.append({"h0": h0c, "qk0": qk0c, "v0": v0c})
    return shared, per_core, (tuple(ln_trivial), bias_zero)


def assemble_output(core_logits):
    """core_logits: list of [V, NTOK] fp32 -> [B, T, V]."""
    out = np.empty((B, T, V), np.float32)
    for core in range(NCORES):
        lg = core_logits[core].reshape(V, SEQ, T)
        out[SEQ * core:SEQ * core + SEQ] = lg.transpose(1, 2, 0)
    return out


def get_program(flags):
    ln_trivial, bias_zero = flags
    key = (ln_trivial, bias_zero)
    if key not in _PROGRAM_CACHE:
        _PROGRAM_CACHE[key] = build_program(list(ln_trivial), bias_zero)
    return _PROGRAM_CACHE[key]


def reset_device():
    """Recover a wedged accelerator (axon session reset). Best-effort."""
    try:
        import ctypes
        import jax
        jax.devices()
        lib = ctypes.CDLL('/opt/axon/libaxon_pjrt.so')
        lib.axon_reset.restype = ctypes.c_int64
        lib.axon_reset()
    except Exception:
        pass


def kernel(**inputs):
    from concourse.bass_utils import run_bass_kernel_spmd
    shared, per_core, flags = prepare_inputs(inputs)
    nc = get_program(flags)
    in_maps = [dict(shared, **per_core[c]) for c in range(NCORES)]
    try:
        res = run_bass_kernel_spmd(nc, in_maps, core_ids=list(range(NCORES)))
    except Exception:
        # A previous (profiled) session can leave the device wedged; reset
        # the axon session and retry once.
        reset_device()
        res = run_bass_kernel_spmd(nc, in_maps, core_ids=list(range(NCORES)))
    return assemble_output([res.results[c]["logits"] for c in range(NCORES)])


# revision 25
# speedup vs baseline: 1.1258x; 1.1258x over previous
"""Trainium2 Bass kernel for nn_AutoregressiveArithmeticTransformer.

6-layer dense transformer: B=16, T=512, E=512, NH=8 heads x HS=64, FF=2048,
V=16, causal attention, pre-LN, learned abacus embedding, logits / 0.8.

Strategy: data-parallel over batch across 8 NeuronCores (2 sequences per
core, no collectives). Activations live feature-major in SBUF
([E-partitions, tokens]); the residual stream is bf16; weights are streamed
per-layer in bf16; all matmuls run in bf16 with fp32 PSUM accumulation.

v2 over the original baseline:
  - layer-0 LN1 + Q/K/V projections precomputed on the host (the embedding
    gather is already host-side); q0/k0/v0 are DMA'd directly.
  - softmax denominator ones-matmuls col-tiled (M=64 pairs at tile
    positions (0,0)/(0,64)) so the two heads of a pair run concurrently
    in the PE array; same for the attention-V matmuls (as before).
  - score pairs land in one 2-bank PSUM tile so exp is ONE activation op
    per key-chunk ([128, 2, njw]); reciprocal and the o*(1/den) multiply
    are one [128,512] op per (seq, head-pair).
  - q/k PSUM results share a 2-bank tile -> single cast per (hp, tt);
    V-projection copies merged in pairs; FFN1 relu merged in pairs.
  - bf16 residual stream: no fp32->bf16 cast before LN stats matmuls.
  - LN apply multiplies and causal tri-mask multiplies run on the
    otherwise-idle GpSimd engine.
"""

import numpy as np
import ml_dtypes

import concourse.bacc as bacc
import concourse.tile as tile
from concourse import mybir

F32 = mybir.dt.float32
BF16 = mybir.dt.bfloat16
AF = mybir.ActivationFunctionType
OP = mybir.AluOpType

# Model constants (hardcoded per contest contract)
V, E, NH, HS, FF, NB, L = 16, 512, 8, 64, 2048, 6, 512
B, T = 16, 512
TEMP = 1.0 * 0.8
EPS = 1e-5
SCALE = HS ** -0.5  # 0.125

NCORES = 8
SEQ = 2              # sequences per core
NTOK = SEQ * T       # 1024 tokens per core
C = E // 128         # 4 E-chunks
CF = FF // 128       # 16 FF-chunks
HP = NH // 2         # 4 head-pairs
NJ = T // 128        # 4 tk chunks per sequence

_PROGRAM_CACHE = {}


def _emit_ln_tt(nc, pools, h_t, ones_t, eps2_t, g_ap, b_ap, trivial, tt, xn):
    """One token-tile of LayerNorm from bf16 h_t into caller-alloc'd xn.

    Chain is kept DVE-local with a single ACT hop (Sqrt):
      mu = s1/E; var = s2/E - mu^2; sig = sqrt(var + eps); r = 1/sig;
      xn = h*r - mu*r.
    """
    stats, stats_bf = pools["stats"], pools["stats_bf"]
    ps1 = pools["ps1"]
    sq = pools["sq"]
    sl = slice(tt * 512, tt * 512 + 512)
    s1 = ps1.tile([128, 512], F32, tag="ps1")
    s2 = ps1.tile([128, 512], F32, tag="ps1")
    for c in range(C):
        nc.vector.tensor_tensor(sq[:, c, sl], h_t[:, c, sl], h_t[:, c, sl],
                                OP.mult)
        nc.tensor.matmul(s1[:], ones_t[:], h_t[:, c, sl],
                         start=(c == 0), stop=(c == C - 1))
        nc.tensor.matmul(s2[:], ones_t[:], sq[:, c, sl],
                         start=(c == 0), stop=(c == C - 1))
    mu = stats.tile([128, 512], F32, tag="stats")
    nc.vector.tensor_scalar(out=mu[:], in0=s1[:],
                            scalar1=1.0 / float(E), scalar2=None, op0=OP.mult)
    msq = stats.tile([128, 512], F32, tag="stats")
    nc.vector.tensor_tensor(msq[:], mu[:], mu[:], OP.mult)
    var = stats.tile([128, 512], F32, tag="stats")
    nc.vector.scalar_tensor_tensor(out=var[:], in0=s2[:],
                                   scalar=1.0 / float(E), in1=msq[:],
                                   op0=OP.mult, op1=OP.subtract)
    std = stats.tile([128, 512], F32, tag="stats")
    nc.scalar.activation(std[:], var[:], AF.Sqrt, bias=eps2_t[:])
    rc = stats.tile([128, 512], F32, tag="stats")
    nc.vector.reciprocal_approx_fast(out=rc[:], in_=std[:])
    r_bf = stats_bf.tile([128, 512], BF16, tag="r_bf")
    nc.scalar.copy(r_bf[:], rc[:])
    z_bf = stats_bf.tile([128, 512], BF16, tag="z_bf")
    nc.vector.tensor_tensor(z_bf[:], mu[:], rc[:], OP.mult)
    if xn is not None:
        for c in range(C):
            nc.vector.tensor_tensor(xn[:, c, sl], h_t[:, c, sl], r_bf[:],
                                    OP.mult)
            nc.vector.tensor_tensor(xn[:, c, sl], xn[:, c, sl], z_bf[:],
                                    OP.subtract)
            if not trivial:
                nc.vector.tensor_scalar(out=xn[:, c, sl], in0=xn[:, c, sl],
                                        scalar1=g_ap[:, c:c + 1],
                                        scalar2=b_ap[:, c:c + 1],
                                        op0=OP.mult, op1=OP.add)
    return r_bf, z_bf


def _alloc_xn(pools):
    return pools["scr"].tile([128, C, NTOK], BF16, tag="scratch", name="xnt")


def _emit_ln(nc, pools, h_t, ones_t, eps2_t, g_ap, b_ap, trivial):
    xn = _alloc_xn(pools)
    rz = []
    for tt in range(2):
        rz.append(_emit_ln_tt(nc, pools, h_t, ones_t, eps2_t, g_ap, b_ap,
                              trivial, tt, xn))
    return xn, rz


def build_program(ln_trivial, bias_zero, nb_run=NB, ln_general_params=True):
    """Build the Bass program.

    ln_trivial: list of NB*2+1 bools (ln1/ln2 per layer then lnf); when True
    the g/b application op is skipped.  bias_zero: (pb, fb1, fb2) all-zero
    flags enabling merged residual/relu fast paths."""
    import os
    corr_en = os.environ.get("KERNEL_CORR", "1") == "1"
    pbz, fb1z, fb2z = bias_zero
    nc = bacc.Bacc(None, target_bir_lowering=False)

    h0_d = nc.dram_tensor("h0", [128, C * NTOK], BF16, kind="ExternalInput")
    qk0_d = nc.dram_tensor("qk0", [128, HP * 2 * NTOK], BF16,
                           kind="ExternalInput")
    v0_d = nc.dram_tensor("v0", [128, SEQ * NJ * 512], BF16,
                          kind="ExternalInput")
    wq_d = nc.dram_tensor("wq", [NB, 128, C * 512], BF16, kind="ExternalInput")
    wk_d = nc.dram_tensor("wk", [NB, 128, C * 512], BF16, kind="ExternalInput")
    wv_d = nc.dram_tensor("wv", [NB, 128, C * 512], BF16, kind="ExternalInput")
    pw_d = nc.dram_tensor("pw", [NB, 128, C * 512], BF16, kind="ExternalInput")
    f1_d = nc.dram_tensor("f1", [NB, 128, C * FF], BF16, kind="ExternalInput")
    f2_d = nc.dram_tensor("f2", [NB, 128, CF * 512], BF16, kind="ExternalInput")
    pb_d = nc.dram_tensor("pb", [128, NB * C], F32, kind="ExternalInput")
    fb1_d = nc.dram_tensor("fb1", [128, NB * CF], F32, kind="ExternalInput")
    fb2_d = nc.dram_tensor("fb2", [128, NB * C], F32, kind="ExternalInput")
    ow_d = nc.dram_tensor("ow", [128, C * V], BF16, kind="ExternalInput")
    ob_d = nc.dram_tensor("ob", [V, 1], F32, kind="ExternalInput")
    tri_d = nc.dram_tensor("tri", [128, 128], BF16, kind="ExternalInput")
    # negated column sums for LN-corrected first blocks
    csqk_d = nc.dram_tensor("csqk", [128, NB * 2], F32, kind="ExternalInput")
    csf1_d = nc.dram_tensor("csf1", [128, NB * 4], F32, kind="ExternalInput")
    csow_d = nc.dram_tensor("csow", [V, 1], F32, kind="ExternalInput")
    lng_d = lnb_d = None
    if ln_general_params:
        lng_d = nc.dram_tensor("lng", [128, (2 * NB + 1) * C], F32,
                               kind="ExternalInput")
        lnb_d = nc.dram_tensor("lnb", [128, (2 * NB + 1) * C], F32,
                               kind="ExternalInput")
    out_d = nc.dram_tensor("logits", [V, NTOK], F32, kind="ExternalOutput")

    from contextlib import ExitStack
    with ExitStack() as ctx:
        tc = ctx.enter_context(tile.TileContext(nc))
        consts = ctx.enter_context(tc.tile_pool(name="consts", bufs=1))
        hpool = ctx.enter_context(tc.tile_pool(name="hpool", bufs=1))
        wqkv = ctx.enter_context(tc.tile_pool(name="wqkv", bufs=1))
        wff1 = ctx.enter_context(tc.tile_pool(name="wff1", bufs=1))
        wff2 = ctx.enter_context(tc.tile_pool(name="wff2", bufs=1))
        scr = ctx.enter_context(tc.tile_pool(name="scr", bufs=3))
        sqpool = ctx.enter_context(tc.tile_pool(name="sqp", bufs=1))
        qk = ctx.enter_context(tc.tile_pool(name="qk", bufs=2))
        vt = ctx.enter_context(tc.tile_pool(name="vt", bufs=1))
        pp = ctx.enter_context(tc.tile_pool(name="pp", bufs=3))
        osb = ctx.enter_context(tc.tile_pool(name="osb", bufs=1))
        ffa = ctx.enter_context(tc.tile_pool(name="ffa", bufs=2))
        corr = ctx.enter_context(tc.tile_pool(name="corr", bufs=2))
        stats = ctx.enter_context(tc.tile_pool(name="stats", bufs=5))
        rdp = ctx.enter_context(tc.tile_pool(name="rdp", bufs=2))
        stats_bf = ctx.enter_context(tc.tile_pool(name="stats_bf", bufs=3))
        ps2 = ctx.enter_context(tc.tile_pool(name="ps2", bufs=3, space="PSUM"))
        ps1 = ctx.enter_context(tc.tile_pool(name="ps1", bufs=2, space="PSUM"))

        sq_t = sqpool.tile([128, C, NTOK], BF16)
        pools = {"scr": scr, "sq": sq_t, "stats": stats, "rdp": rdp,
                 "stats_bf": stats_bf, "ps2": ps2, "ps1": ps1}

        ones_t = consts.tile([128, 128], BF16)
        nc.gpsimd.memset(ones_t[:], 1.0)
        eps2_t = consts.tile([128, 1], F32)
        nc.gpsimd.memset(eps2_t[:], EPS)
        tri_t = consts.tile([128, 128], BF16)
        nc.sync.dma_start(tri_t[:], tri_d[:])
        pb_t = consts.tile([128, NB * C], F32)
        nc.sync.dma_start(pb_t[:], pb_d[:])
        fb1_t = consts.tile([128, NB * CF], F32)
        nc.sync.dma_start(fb1_t[:], fb1_d[:])
        fb2_t = consts.tile([128, NB * C], F32)
        nc.sync.dma_start(fb2_t[:], fb2_d[:])
        ow_t = consts.tile([128, C, V], BF16)
        nc.sync.dma_start(ow_t[:], ow_d[:].rearrange("p (c v) -> p c v", v=V))
        ob_t = consts.tile([V, 1], F32)
        nc.sync.dma_start(ob_t[:], ob_d[:])
        csqk_t = consts.tile([128, NB, 2], F32)
        nc.sync.dma_start(csqk_t[:], csqk_d[:].rearrange(
            "p (l u) -> p l u", u=2))
        csf1_t = consts.tile([128, NB, 4], F32)
        nc.sync.dma_start(csf1_t[:], csf1_d[:].rearrange(
            "p (l u) -> p l u", u=4))
        csow_t = consts.tile([V, 1], F32)
        nc.sync.dma_start(csow_t[:], csow_d[:])
        lng_t = lnb_t = None
        if ln_general_params:
            lng_t = consts.tile([128, 2 * NB + 1, C], F32)
            nc.sync.dma_start(lng_t[:], lng_d[:].rearrange(
                "p (l c) -> p l c", c=C))
            lnb_t = consts.tile([128, 2 * NB + 1, C], F32)
            nc.sync.dma_start(lnb_t[:], lnb_d[:].rearrange(
                "p (l c) -> p l c", c=C))

        h_t = hpool.tile([128, C, NTOK], BF16)
        nc.sync.dma_start(h_t[:], h0_d[:].rearrange(
            "p (c t) -> p c t", t=NTOK))

        def ln_params(idx):
            if ln_general_params and not ln_trivial[idx]:
                return lng_t[:, idx, :], lnb_t[:, idx, :], False
            return None, None, True

        for i in range(nb_run):
            # ---- load this layer's weights ----
            if i > 0:
                wq_t = wqkv.tile([128, C, 512], BF16, tag="wq")
                nc.sync.dma_start(wq_t[:], wq_d[i].rearrange(
                    "p (c m) -> p c m", m=512))
                wk_t = wqkv.tile([128, C, 512], BF16, tag="wk")
                nc.sync.dma_start(wk_t[:], wk_d[i].rearrange(
                    "p (c m) -> p c m", m=512))
                wv_t = wqkv.tile([128, C, 512], BF16, tag="wv")
                nc.sync.dma_start(wv_t[:], wv_d[i].rearrange(
                    "p (c m) -> p c m", m=512))
            pw_t = wqkv.tile([128, C, 512], BF16, tag="pw")
            nc.sync.dma_start(pw_t[:], pw_d[i].rearrange(
                "p (c m) -> p c m", m=512))
            f1_t = wff1.tile([128, C, FF], BF16, tag="f1")
            nc.sync.dma_start(f1_t[:], f1_d[i].rearrange(
                "p (c m) -> p c m", m=FF))
            f2_t = wff2.tile([128, CF, 512], BF16, tag="f2")
            nc.sync.dma_start(f2_t[:], f2_d[i].rearrange(
                "p (c m) -> p c m", m=512))

            # ---- LN1 output for this layer (layer 0: host-computed;
            #      others peeled into the previous layer's FFN emission) ----
            xn = xn_next if i > 0 else None
            qk_corr = corr_en and i > 0 and ln_trivial[2 * i]

            # ---- corrected Q/K for head-pair 0: matmuls run on h_t so the
            #      PE has work while the LN1 chain computes r/z; the drain
            #      applies q = (Wq^T h)*r - z*colsum(Wq). ----
            qk_hp0 = None
            if qk_corr:
                qk_hp0 = qk.tile([128, 2, NTOK], BF16, tag="qk")
                msl0 = slice(0, 128)
                for tt in range(2):
                    r1, z1 = rz1_next[tt]
                    sl = slice(tt * 512, tt * 512 + 512)
                    qkp = ps2.tile([128, 2, 512], F32, tag="ps2")
                    for c in range(C):
                        nc.tensor.matmul(qkp[:, 0, :], wq_t[:, c, msl0],
                                         h_t[:, c, sl],
                                         start=(c == 0), stop=(c == C - 1))
                        nc.tensor.matmul(qkp[:, 1, :], wk_t[:, c, msl0],
                                         h_t[:, c, sl],
                                         start=(c == 0), stop=(c == C - 1))
                    nc.vector.tensor_tensor(
                        qk_hp0[:, :, sl], qkp[:],
                        r1[:, None, :].to_broadcast((128, 2, 512)),
                        OP.mult)
                    for u in range(2):
                        nc.vector.scalar_tensor_tensor(
                            out=qk_hp0[:, u, sl], in0=z1[:],
                            scalar=csqk_t[:, i, u:u + 1],
                            in1=qk_hp0[:, u, sl], op0=OP.mult, op1=OP.add)

            # ---- V, token-major: vt[tk, hd*64+d] ----
            vt_t = vt.tile([128, SEQ * NJ, 512], BF16, tag="vt")
            if i == 0:
                nc.sync.dma_start(vt_t[:], v0_d[:].rearrange(
                    "p (g m) -> p g m", m=512))
            else:
                for jp in range(SEQ * NJ // 2):
                    vp = ps2.tile([128, 2, 512], F32, tag="ps2")
                    for u in range(2):
                        jg = jp * 2 + u
                        for c in range(C):
                            nc.tensor.matmul(
                                vp[:, u, :],
                                xn[:, c, jg * 128:(jg + 1) * 128],
                                wv_t[:, c, :],
                                start=(c == 0), stop=(c == C - 1))
                    nc.scalar.copy(vt_t[:, jp * 2:jp * 2 + 2, :], vp[:])

            o_t = osb.tile([128, C, NTOK], BF16, tag="o")

            def emit_den_o(s, hp, p_t):
                base = s * T
                den = ps1.tile([128, 512], F32, tag="ps1")
                for j in range(NJ):
                    off = j * 128
                    njw = T - off
                    for h2 in range(2):
                        nc.tensor.matmul(den[h2 * 64:h2 * 64 + 64, off:512],
                                         ones_t[:, 0:64],
                                         p_t[:, h2, j, 0:njw],
                                         start=(j == 0), stop=(j == NJ - 1),
                                         skip_group_check=True)
                rd = rdp.tile([128, 512], F32, tag="rd")
                nc.vector.reciprocal_approx_fast(out=rd[:], in_=den[:])
                op_ps = ps1.tile([128, 512], F32, tag="ps1")
                for j in range(NJ):
                    off = j * 128
                    njw = T - off
                    for h2 in range(2):
                        head = hp * 2 + h2
                        nc.tensor.matmul(
                            op_ps[h2 * 64:h2 * 64 + 64, off:T],
                            vt_t[:, s * NJ + j, head * 64:head * 64 + 64],
                            p_t[:, h2, j, 0:njw],
                            start=(j == 0), stop=(j == NJ - 1),
                            skip_group_check=True)
                nc.vector.tensor_tensor(o_t[:, hp, base:base + T],
                                        op_ps[:, 0:T], rd[:], OP.mult)

            pending = None
            for hp in range(HP):
                msl = slice(hp * 128, (hp + 1) * 128)
                if hp == 0 and qk_hp0 is not None:
                    qk_t = qk_hp0
                else:
                    qk_t = qk.tile([128, 2, NTOK], BF16, tag="qk")
                    if i == 0:
                        nc.sync.dma_start(
                            qk_t[:],
                            qk0_d[:, hp * 2 * NTOK:(hp + 1) * 2 * NTOK]
                            .rearrange("p (q t) -> p q t", t=NTOK))
                    else:
                        for tt in range(2):
                            sl = slice(tt * 512, tt * 512 + 512)
                            qkp = ps2.tile([128, 2, 512], F32, tag="ps2")
                            for c in range(C):
                                nc.tensor.matmul(qkp[:, 0, :],
                                                 wq_t[:, c, msl],
                                                 xn[:, c, sl],
                                                 start=(c == 0),
                                                 stop=(c == C - 1))
                                nc.tensor.matmul(qkp[:, 1, :],
                                                 wk_t[:, c, msl],
                                                 xn[:, c, sl],
                                                 start=(c == 0),
                                                 stop=(c == C - 1))
                            nc.scalar.copy(qk_t[:, :, sl], qkp[:])

                for s in range(SEQ):
                    base = s * T
                    p_t = pp.tile([128, 2, NJ, 512], BF16, tag="p")
                    for j in range(NJ):
                        off = j * 128
                        njw = T - off
                        sT = ps2.tile([128, 2, 512], F32, tag="ps2")
                        for h2 in range(2):
                            dsl = slice(h2 * 64, h2 * 64 + 64)
                            nc.tensor.matmul(
                                sT[:, h2, 0:njw],
                                qk_t[dsl, 1, base + off:base + off + 128],
                                qk_t[dsl, 0, base + off:base + T],
                                start=True, stop=True)
                        nc.scalar.activation(
                            p_t[:, :, j, 0:njw], sT[:, :, 0:njw],
                            AF.Exp, scale=SCALE)
                        nc.vector.tensor_tensor(
                            p_t[:, :, j, 0:128], p_t[:, :, j, 0:128],
                            tri_t[:, None, :].to_broadcast(
                                (128, 2, 128)), OP.mult)
                    if pending is not None:
                        emit_den_o(*pending)
                    pending = (s, hp, p_t)
            emit_den_o(*pending)

            # ---- attention out projection + residual ----
            for tt in range(2):
                sl = slice(tt * 512, tt * 512 + 512)
                for mcp in range(C // 2):
                    pj = ps2.tile([128, 2, 512], F32, tag="ps2")
                    for u in range(2):
                        mc = mcp * 2 + u
                        for c in range(C):
                            nc.tensor.matmul(
                                pj[:, u, :],
                                pw_t[:, c, mc * 128:(mc + 1) * 128],
                                o_t[:, c, sl],
                                start=(c == 0), stop=(c == C - 1))
                    if pbz:
                        nc.vector.tensor_tensor(
                            h_t[:, mcp * 2:mcp * 2 + 2, sl], pj[:],
                            h_t[:, mcp * 2:mcp * 2 + 2, sl], OP.add)
                    else:
                        for u in range(2):
                            mc = mcp * 2 + u
                            nc.vector.scalar_tensor_tensor(
                                out=h_t[:, mc, sl], in0=pj[:, u, :],
                                scalar=pb_t[:, i * C + mc:i * C + mc + 1],
                                in1=h_t[:, mc, sl], op0=OP.add, op1=OP.add)

            # ---- LN2 + FFN (token-tile split) ----
            g_ap, b_ap, triv = ln_params(2 * i + 1)
            ffn_corr = corr_en and triv
            xn2 = _alloc_xn(pools)
            r2_0, z2_0 = _emit_ln_tt(nc, pools, h_t, ones_t, eps2_t,
                                     g_ap, b_ap, triv, 0, xn2)
            # corrected first FFN1 blocks (tt=0): matmuls on h_t fill the PE
            # while the LN2 chain runs; drain applies r/z + colsum correction.
            corr_fa = []
            if ffn_corr:
                for mfp in range(2):
                    fp = ps2.tile([128, 2, 512], F32, tag="ps2")
                    for u in range(2):
                        mf = mfp * 2 + u
                        for c in range(C):
                            nc.tensor.matmul(
                                fp[:, u, :],
                                f1_t[:, c, mf * 128:(mf + 1) * 128],
                                h_t[:, c, 0:512],
                                start=(c == 0), stop=(c == C - 1))
                    wtmp = corr.tile([128, 2, 512], F32, tag="corr")
                    nc.vector.tensor_tensor(
                        wtmp[:], fp[:],
                        r2_0[:, None, :].to_broadcast((128, 2, 512)), OP.mult)
                    for u in range(2):
                        mf = mfp * 2 + u
                        nc.vector.scalar_tensor_tensor(
                            out=wtmp[:, u, :], in0=z2_0[:],
                            scalar=csf1_t[:, i, mf:mf + 1],
                            in1=wtmp[:, u, :], op0=OP.mult, op1=OP.add)
                    corr_fa.append(wtmp)
            _emit_ln_tt(nc, pools, h_t, ones_t, eps2_t, g_ap, b_ap, triv,
                        1, xn2)

            for tt in range(2):
                sl = slice(tt * 512, tt * 512 + 512)
                fa = ffa.tile([128, CF, 512], BF16, tag="fa")
                for mfp in range(CF // 2):
                    if tt == 0 and ffn_corr and mfp < 2:
                        src = corr_fa[mfp][:]
                    else:
                        fp = ps2.tile([128, 2, 512], F32, tag="ps2")
                        for u in range(2):
                            mf = mfp * 2 + u
                            for c in range(C):
                                nc.tensor.matmul(
                                    fp[:, u, :],
                                    f1_t[:, c, mf * 128:(mf + 1) * 128],
                                    xn2[:, c, sl],
                                    start=(c == 0), stop=(c == C - 1))
                        src = fp[:]
                    if fb1z:
                        nc.scalar.activation(
                            fa[:, mfp * 2:mfp * 2 + 2, :], src, AF.Relu)
                    else:
                        for u in range(2):
                            mf = mfp * 2 + u
                            nc.scalar.activation(
                                fa[:, mf, :], src[:, u, :], AF.Relu,
                                bias=fb1_t[:, i * CF + mf:i * CF + mf + 1])
                for mcp in range(C // 2):
                    f2p = ps2.tile([128, 2, 512], F32, tag="ps2")
                    for u in range(2):
                        for c16 in range(CF):
                            nc.tensor.matmul(
                                f2p[:, u, :],
                                f2_t[:, c16,
                                     (mcp * 2 + u) * 128:
                                     (mcp * 2 + u + 1) * 128],
                                fa[:, c16, :],
                                start=(c16 == 0), stop=(c16 == CF - 1))
                    if fb2z:
                        nc.vector.tensor_tensor(
                            h_t[:, mcp * 2:mcp * 2 + 2, sl], f2p[:],
                            h_t[:, mcp * 2:mcp * 2 + 2, sl], OP.add)
                    else:
                        for u in range(2):
                            mc = mcp * 2 + u
                            nc.vector.scalar_tensor_tensor(
                                out=h_t[:, mc, sl], in0=f2p[:, u, :],
                                scalar=fb2_t[:, i * C + mc:i * C + mc + 1],
                                in1=h_t[:, mc, sl], op0=OP.add, op1=OP.add)
                # peel next layer's LN1(tt) here so its scalar/vector chain
                # hides behind the other token-tile's FFN matmuls
                if i + 1 < nb_run:
                    if tt == 0:
                        xn_next = _alloc_xn(pools)
                        rz1_next = []
                    g_ap, b_ap, triv = ln_params(2 * (i + 1))
                    rz1_next.append(_emit_ln_tt(nc, pools, h_t, ones_t,
                                                eps2_t, g_ap, b_ap, triv,
                                                tt, xn_next))

        # ---- final LN + logits (corrected: logits matmuls run on h_t) ----
        g_ap, b_ap, triv = (ln_params(2 * NB) if nb_run == NB
                            else (None, None, True))
        lg_sb = consts.tile([V, NTOK], F32)
        if triv and corr_en:
            rzf = [_emit_ln_tt(nc, pools, h_t, ones_t, eps2_t, g_ap, b_ap,
                               triv, tt, None) for tt in range(2)]
            for tt in range(2):
                sl = slice(tt * 512, tt * 512 + 512)
                lg = ps1.tile([V, 512], F32, tag="ps1")
                for c in range(C):
                    nc.tensor.matmul(lg[:], ow_t[:, c, :], h_t[:, c, sl],
                                     start=(c == 0), stop=(c == C - 1))
                rf, zf = rzf[tt]
                w16 = rdp.tile([V, 512], F32, tag="w16")
                nc.vector.tensor_tensor(w16[:], lg[:], rf[0:V, :], OP.mult)
                nc.vector.scalar_tensor_tensor(
                    out=w16[:], in0=zf[0:V, :], scalar=csow_t[:],
                    in1=w16[:], op0=OP.mult, op1=OP.add)
                nc.vector.tensor_scalar_add(lg_sb[:, sl], w16[:], ob_t[:])
        else:
            xnf, _ = _emit_ln(nc, pools, h_t, ones_t, eps2_t, g_ap, b_ap,
                              triv)
            for tt in range(2):
                sl = slice(tt * 512, tt * 512 + 512)
                lg = ps1.tile([V, 512], F32, tag="ps1")
                for c in range(C):
                    nc.tensor.matmul(lg[:], ow_t[:, c, :], xnf[:, c, sl],
                                     start=(c == 0), stop=(c == C - 1))
                nc.vector.tensor_scalar_add(lg_sb[:, sl], lg[:], ob_t[:])
        nc.sync.dma_start(out_d[:], lg_sb[:])

    nc.finalize()
    return nc


def prepare_inputs(inputs):
    """Host-side preprocessing: embedding gather, layer-0 LN1+QKV, weight
    layout + bf16 cast.  Returns (shared_map, per_core_maps, flags)."""
    f32 = np.float32
    bf16 = ml_dtypes.bfloat16
    x = np.asarray(inputs["x"]).astype(np.int64)
    emb = np.asarray(inputs["emb"], dtype=f32)
    pos = np.asarray(inputs["pos"], dtype=f32)

    positions = np.minimum(np.arange(T), L - 1)
    h0 = (emb[x] + pos[positions][None, :, :]).astype(bf16).astype(f32)

    # layer-0 LN1 + Q/K/V on host (fp32, then bf16)
    g1 = np.asarray(inputs["ln1_g"][0], dtype=f32)
    b1 = np.asarray(inputs["ln1_b"][0], dtype=f32)
    mu = h0.mean(-1, keepdims=True)
    var = np.square(h0 - mu).mean(-1, keepdims=True)
    xn0 = ((h0 - mu) / np.sqrt(var + EPS) * g1 + b1).astype(bf16).astype(f32)
    wq0 = np.asarray(inputs["wq"][0], dtype=f32).astype(bf16).astype(f32)
    wk0 = np.asarray(inputs["wk"][0], dtype=f32).astype(bf16).astype(f32)
    wv0 = np.asarray(inputs["wv"][0], dtype=f32).astype(bf16).astype(f32)
    # [B, T, NH*HS] with head-major feature order
    q0 = np.einsum('bte,hed->bthd', xn0, wq0).reshape(B, T, NH * HS)
    k0 = np.einsum('bte,hed->bthd', xn0, wk0).reshape(B, T, NH * HS)
    v0 = np.einsum('bte,hed->bthd', xn0, wv0).reshape(B, T, NH * HS)

    def to_dev_lhst(mat, kchunks, mcols):
        m = np.ascontiguousarray(mat.astype(bf16))
        return m.reshape(kchunks, 128, mcols).transpose(1, 0, 2).reshape(
            128, kchunks * mcols)

    wq = np.asarray(inputs["wq"], dtype=f32)
    wk = np.asarray(inputs["wk"], dtype=f32)
    wv = np.asarray(inputs["wv"], dtype=f32)
    pw = np.asarray(inputs["proj_w"], dtype=f32)
    f1 = np.asarray(inputs["ff_w1"], dtype=f32)
    f2 = np.asarray(inputs["ff_w2"], dtype=f32)

    wq_dev = np.stack([to_dev_lhst(wq[i].transpose(1, 0, 2).reshape(E, NH * HS),
                                   C, 512) for i in range(NB)])
    wk_dev = np.stack([to_dev_lhst(wk[i].transpose(1, 0, 2).reshape(E, NH * HS),
                                   C, 512) for i in range(NB)])
    wv_dev = np.stack([to_dev_lhst(wv[i].transpose(1, 0, 2).reshape(E, NH * HS),
                                   C, 512) for i in range(NB)])
    pw_dev = np.stack([to_dev_lhst(pw[i], C, 512) for i in range(NB)])
    f1_dev = np.stack([to_dev_lhst(f1[i], C, FF) for i in range(NB)])
    f2_dev = np.stack([to_dev_lhst(f2[i], CF, 512) for i in range(NB)])

    def vec_dev(v, chunks):
        return np.ascontiguousarray(v.astype(f32).reshape(chunks, 128).T)

    pb = np.asarray(inputs["proj_b"], dtype=f32)
    fb1 = np.asarray(inputs["ff_b1"], dtype=f32)
    fb2 = np.asarray(inputs["ff_b2"], dtype=f32)
    bias_zero = (bool(np.all(pb == 0.0)), bool(np.all(fb1 == 0.0)),
                 bool(np.all(fb2 == 0.0)))
    pb_dev = np.concatenate([vec_dev(pb[i], C) for i in range(NB)], axis=1)
    fb1_dev = np.concatenate([vec_dev(fb1[i], CF) for i in range(NB)], axis=1)
    fb2_dev = np.concatenate([vec_dev(fb2[i], C) for i in range(NB)], axis=1)
    ow_dev = to_dev_lhst(np.asarray(inputs["out_w"], dtype=f32) / TEMP, C, V)
    ob_dev = (np.asarray(inputs["out_b"], dtype=f32) / TEMP).reshape(V, 1)
    tri_dev = np.triu(np.ones((128, 128), dtype=f32)).astype(bf16)

    # negated column sums (of the bf16-cast weights) for corrected blocks
    def neg_cs(mat, cols):
        mb = mat.astype(bf16).astype(f32)
        return -mb[:, cols].sum(axis=0)

    csqk_dev = np.zeros((128, NB * 2), f32)
    csf1_dev = np.zeros((128, NB * 4), f32)
    for i in range(NB):
        wq_flat = wq[i].transpose(1, 0, 2).reshape(E, NH * HS)
        wk_flat = wk[i].transpose(1, 0, 2).reshape(E, NH * HS)
        csqk_dev[:, i * 2 + 0] = neg_cs(wq_flat, slice(0, 128))
        csqk_dev[:, i * 2 + 1] = neg_cs(wk_flat, slice(0, 128))
        for mf in range(4):
            csf1_dev[:, i * 4 + mf] = neg_cs(
                f1[i], slice(mf * 128, (mf + 1) * 128))
    csow_dev = np.ascontiguousarray(
        neg_cs(np.asarray(inputs["out_w"], dtype=f32) / TEMP,
               slice(0, V)).reshape(V, 1))

    gs, bs, ln_trivial = [], [], []
    for i in range(NB):
        for nm_g, nm_b in (("ln1_g", "ln1_b"), ("ln2_g", "ln2_b")):
            g = np.asarray(inputs[nm_g][i], dtype=f32)
            b = np.asarray(inputs[nm_b][i], dtype=f32)
            gs.append(vec_dev(g, C))
            bs.append(vec_dev(b, C))
            ln_trivial.append(bool(np.all(g == 1.0) and np.all(b == 0.0)))
    g = np.asarray(inputs["lnf_g"], dtype=f32)
    b = np.asarray(inputs["lnf_b"], dtype=f32)
    gs.append(vec_dev(g, C))
    bs.append(vec_dev(b, C))
    ln_trivial.append(bool(np.all(g == 1.0) and np.all(b == 0.0)))
    lng_dev = np.concatenate(gs, axis=1)
    lnb_dev = np.concatenate(bs, axis=1)

    shared = {
        "wq": wq_dev, "wk": wk_dev, "wv": wv_dev, "pw": pw_dev,
        "f1": f1_dev, "f2": f2_dev, "pb": pb_dev, "fb1": fb1_dev,
        "fb2": fb2_dev, "ow": ow_dev, "ob": ob_dev, "tri": tri_dev,
        "lng": lng_dev, "lnb": lnb_dev, "csqk": csqk_dev, "csf1": csf1_dev,
        "csow": csow_dev,
    }

    per_core = []
    for core in range(NCORES):
        csl = slice(SEQ * core, SEQ * core + SEQ)

        def featmaj(a):                      # [SEQ, T, F] -> [128, F/128*NTOK]
            fT = a[csl].transpose(2, 0, 1).reshape(-1, NTOK)   # [F, NTOK]
            ch = fT.shape[0] // 128
            return np.ascontiguousarray(
                fT.reshape(ch, 128, NTOK).transpose(1, 0, 2).reshape(
                    128, ch * NTOK).astype(bf16))

        h0c = featmaj(h0)                       # [128, C*NTOK]
        # qk0: [128, hp, {q,k}, NTOK]; partition = h2*64+d of the pair
        qf = q0[csl].transpose(2, 0, 1).reshape(NH * HS, NTOK)  # [512, NTOK]
        kf = k0[csl].transpose(2, 0, 1).reshape(NH * HS, NTOK)
        qk0c = np.empty((128, HP, 2, NTOK), dtype=f32)
        for hp in range(HP):
            qk0c[:, hp, 0] = qf[hp * 128:(hp + 1) * 128]
            qk0c[:, hp, 1] = kf[hp * 128:(hp + 1) * 128]
        qk0c = np.ascontiguousarray(
            qk0c.reshape(128, HP * 2 * NTOK).astype(bf16))
        # v0: token-major [128, SEQ*NJ, 512]
        vtok = v0[csl].reshape(NTOK, NH * HS)          # [NTOK, 512]
        v0c = np.ascontiguousarray(
            vtok.reshape(SEQ * NJ, 128, NH * HS).transpose(1, 0, 2).reshape(
                128, SEQ * NJ * 512).astype(bf16))
        per_core.append({"h0": h0c, "qk0": qk0c, "v0": v0c})
    return shared, per_core, (tuple(ln_trivial), bias_zero)


def assemble_output(core_logits):
    """core_logits: list of [V, NTOK] fp32 -> [B, T, V]."""
    out = np.empty((B, T, V), np.float32)
    for core in range(NCORES):
        lg = core_logits[core].reshape(V, SEQ, T)
        out[SEQ * core:SEQ * core + SEQ] = lg.transpose(1, 2, 0)
    return out


def get_program(flags):
    ln_trivial, bias_zero = flags
    key = (ln_trivial, bias_zero)
    if key not in _PROGRAM_CACHE:
        _PROGRAM_CACHE[key] = build_program(list(ln_trivial), bias_zero)
    return _PROGRAM_CACHE[key]


def reset_device():
    """Recover a wedged accelerator (axon session reset). Best-effort."""
    try:
        import ctypes
        import jax
        jax.devices()
        lib = ctypes.CDLL('/opt/axon/libaxon_pjrt.so')
        lib.axon_reset.restype = ctypes.c_int64
        lib.axon_reset()
    except Exception:
        pass


def kernel(**inputs):
    from concourse.bass_utils import run_bass_kernel_spmd
    shared, per_core, flags = prepare_inputs(inputs)
    nc = get_program(flags)
    in_maps = [dict(shared, **per_core[c]) for c in range(NCORES)]
    try:
        res = run_bass_kernel_spmd(nc, in_maps, core_ids=list(range(NCORES)))
    except Exception:
        # A previous (profiled) session can leave the device wedged; reset
        # the axon session and retry once.
        reset_device()
        res = run_bass_kernel_spmd(nc, in_maps, core_ids=list(range(NCORES)))
    return assemble_output([res.results[c]["logits"] for c in range(NCORES)])


# revision 34
# speedup vs baseline: 1.1959x; 1.0623x over previous
"""Trainium2 Bass kernel for nn_AutoregressiveArithmeticTransformer.

6-layer dense transformer: B=16, T=512, E=512, NH=8 heads x HS=64, FF=2048,
V=16, causal attention, pre-LN, learned abacus embedding, logits / 0.8.

Strategy: data-parallel over batch across 8 NeuronCores (2 sequences per
core, no collectives). Activations live feature-major in SBUF
([E-partitions, tokens]); the residual stream is bf16; weights are streamed
per-layer in bf16; all matmuls run in bf16 with fp32 PSUM accumulation.

v2 over the original baseline:
  - layer-0 LN1 + Q/K/V projections precomputed on the host (the embedding
    gather is already host-side); q0/k0/v0 are DMA'd directly.
  - softmax denominator ones-matmuls col-tiled (M=64 pairs at tile
    positions (0,0)/(0,64)) so the two heads of a pair run concurrently
    in the PE array; same for the attention-V matmuls (as before).
  - score pairs land in one 2-bank PSUM tile so exp is ONE activation op
    per key-chunk ([128, 2, njw]); reciprocal and the o*(1/den) multiply
    are one [128,512] op per (seq, head-pair).
  - q/k PSUM results share a 2-bank tile -> single cast per (hp, tt);
    V-projection copies merged in pairs; FFN1 relu merged in pairs.
  - bf16 residual stream: no fp32->bf16 cast before LN stats matmuls.
  - LN apply multiplies and causal tri-mask multiplies run on the
    otherwise-idle GpSimd engine.
"""

import numpy as np
import ml_dtypes

import concourse.bacc as bacc
import concourse.tile as tile
from concourse import mybir

F32 = mybir.dt.float32
BF16 = mybir.dt.bfloat16
AF = mybir.ActivationFunctionType
OP = mybir.AluOpType

# Model constants (hardcoded per contest contract)
V, E, NH, HS, FF, NB, L = 16, 512, 8, 64, 2048, 6, 512
B, T = 16, 512
TEMP = 1.0 * 0.8
EPS = 1e-5
SCALE = HS ** -0.5  # 0.125

NCORES = 8
SEQ = 2              # sequences per core
NTOK = SEQ * T       # 1024 tokens per core
C = E // 128         # 4 E-chunks
CF = FF // 128       # 16 FF-chunks
HP = NH // 2         # 4 head-pairs
NJ = T // 128        # 4 tk chunks per sequence

_PROGRAM_CACHE = {}


def _emit_ln_tt(nc, pools, h_t, ones_t, eps2_t, g_ap, b_ap, trivial, tt, xn):
    """One token-tile of LayerNorm from bf16 h_t into caller-alloc'd xn.

    Chain is kept DVE-local with a single ACT hop (Sqrt):
      mu = s1/E; var = s2/E - mu^2; sig = sqrt(var + eps); r = 1/sig;
      xn = (h - mu)*r.
    Returns (r_bf, mu_bf) for LN-corrected consumers.
    """
    stats, stats_bf = pools["stats"], pools["stats_bf"]
    ps1 = pools["ps1"]
    sq = pools["sq"]
    sl = slice(tt * 512, tt * 512 + 512)
    s1 = ps1.tile([128, 512], F32, tag="ps1")
    s2 = ps1.tile([128, 512], F32, tag="ps1")
    for c in range(C):
        nc.vector.tensor_tensor(sq[:, c, sl], h_t[:, c, sl], h_t[:, c, sl],
                                OP.mult)
        nc.tensor.matmul(s1[:], ones_t[:], h_t[:, c, sl],
                         start=(c == 0), stop=(c == C - 1))
        nc.tensor.matmul(s2[:], ones_t[:], sq[:, c, sl],
                         start=(c == 0), stop=(c == C - 1))
    mu = stats.tile([128, 512], F32, tag="stats")
    nc.vector.tensor_scalar(out=mu[:], in0=s1[:],
                            scalar1=1.0 / float(E), scalar2=None, op0=OP.mult)
    msq = stats.tile([128, 512], F32, tag="stats")
    nc.vector.tensor_tensor(msq[:], mu[:], mu[:], OP.mult)
    var = stats.tile([128, 512], F32, tag="stats")
    nc.vector.scalar_tensor_tensor(out=var[:], in0=s2[:],
                                   scalar=1.0 / float(E), in1=msq[:],
                                   op0=OP.mult, op1=OP.subtract)
    std = stats.tile([128, 512], F32, tag="stats")
    nc.scalar.activation(std[:], var[:], AF.Sqrt, bias=eps2_t[:])
    rc = stats.tile([128, 512], F32, tag="stats")
    nc.vector.reciprocal_approx_fast(out=rc[:], in_=std[:])
    r_bf = stats_bf.tile([128, 512], BF16, tag="r_bf")
    nc.scalar.copy(r_bf[:], rc[:])
    mu_bf = stats_bf.tile([128, 512], BF16, tag="mu_bf")
    nc.vector.tensor_scalar(out=mu_bf[:], in0=mu[:], scalar1=1.0,
                            scalar2=None, op0=OP.mult)
    if xn is not None:
        for c in range(C):
            nc.vector.tensor_tensor(xn[:, c, sl], h_t[:, c, sl], mu_bf[:],
                                    OP.subtract)
            nc.vector.tensor_tensor(xn[:, c, sl], xn[:, c, sl], r_bf[:],
                                    OP.mult)
            if not trivial:
                nc.vector.tensor_scalar(out=xn[:, c, sl], in0=xn[:, c, sl],
                                        scalar1=g_ap[:, c:c + 1],
                                        scalar2=b_ap[:, c:c + 1],
                                        op0=OP.mult, op1=OP.add)
    return r_bf, mu_bf


def _alloc_xn(pools):
    return pools["scr"].tile([128, C, NTOK], BF16, tag="scratch", name="xnt")


def _emit_ln(nc, pools, h_t, ones_t, eps2_t, g_ap, b_ap, trivial):
    xn = _alloc_xn(pools)
    rz = []
    for tt in range(2):
        rz.append(_emit_ln_tt(nc, pools, h_t, ones_t, eps2_t, g_ap, b_ap,
                              trivial, tt, xn))
    return xn, rz


def build_program(ln_trivial, bias_zero, nb_run=NB, ln_general_params=True):
    """Build the Bass program.

    ln_trivial: list of NB*2+1 bools (ln1/ln2 per layer then lnf); when True
    the g/b application op is skipped.  bias_zero: (pb, fb1, fb2) all-zero
    flags enabling merged residual/relu fast paths."""
    import os
    corr_en = os.environ.get("KERNEL_CORR", "1") == "1"
    pbz, fb1z, fb2z = bias_zero
    nc = bacc.Bacc(None, target_bir_lowering=False)

    h0_d = nc.dram_tensor("h0", [128, C * NTOK], BF16, kind="ExternalInput")
    qk0_d = nc.dram_tensor("qk0", [128, HP * 2 * NTOK], BF16,
                           kind="ExternalInput")
    v0_d = nc.dram_tensor("v0", [128, SEQ * NJ * 512], BF16,
                          kind="ExternalInput")
    wq_d = nc.dram_tensor("wq", [NB, 128, C * 512], BF16, kind="ExternalInput")
    wk_d = nc.dram_tensor("wk", [NB, 128, C * 512], BF16, kind="ExternalInput")
    wv_d = nc.dram_tensor("wv", [NB, 128, C * 512], BF16, kind="ExternalInput")
    pw_d = nc.dram_tensor("pw", [NB, 128, C * 512], BF16, kind="ExternalInput")
    f1_d = nc.dram_tensor("f1", [NB, 128, C * FF], BF16, kind="ExternalInput")
    f2_d = nc.dram_tensor("f2", [NB, 128, CF * 512], BF16, kind="ExternalInput")
    pb_d = nc.dram_tensor("pb", [128, NB * C], F32, kind="ExternalInput")
    fb1_d = nc.dram_tensor("fb1", [128, NB * CF], F32, kind="ExternalInput")
    fb2_d = nc.dram_tensor("fb2", [128, NB * C], F32, kind="ExternalInput")
    ow_d = nc.dram_tensor("ow", [128, C * V], BF16, kind="ExternalInput")
    ob_d = nc.dram_tensor("ob", [V, 1], F32, kind="ExternalInput")
    tri_d = nc.dram_tensor("tri", [128, 128], BF16, kind="ExternalInput")
    # negated column sums for LN-corrected first blocks (row vectors: these
    # are K=1 matmul stationaries accumulating cs (x) z into PSUM)
    csqk_d = nc.dram_tensor("csqk", [1, NB * 2 * 128], BF16,
                            kind="ExternalInput")
    csf1_d = nc.dram_tensor("csf1", [1, NB * 4 * 128], BF16,
                            kind="ExternalInput")
    csow_d = nc.dram_tensor("csow", [1, V], BF16, kind="ExternalInput")
    lng_d = lnb_d = None
    if ln_general_params:
        lng_d = nc.dram_tensor("lng", [128, (2 * NB + 1) * C], F32,
                               kind="ExternalInput")
        lnb_d = nc.dram_tensor("lnb", [128, (2 * NB + 1) * C], F32,
                               kind="ExternalInput")
    out_d = nc.dram_tensor("logits", [V, NTOK], F32, kind="ExternalOutput")

    from contextlib import ExitStack
    with ExitStack() as ctx:
        tc = ctx.enter_context(tile.TileContext(nc))
        consts = ctx.enter_context(tc.tile_pool(name="consts", bufs=1))
        hpool = ctx.enter_context(tc.tile_pool(name="hpool", bufs=1))
        wqkv = ctx.enter_context(tc.tile_pool(name="wqkv", bufs=1))
        wff1 = ctx.enter_context(tc.tile_pool(name="wff1", bufs=1))
        wff2 = ctx.enter_context(tc.tile_pool(name="wff2", bufs=1))
        scr = ctx.enter_context(tc.tile_pool(name="scr", bufs=3))
        sqpool = ctx.enter_context(tc.tile_pool(name="sqp", bufs=1))
        qk = ctx.enter_context(tc.tile_pool(name="qk", bufs=2))
        vt = ctx.enter_context(tc.tile_pool(name="vt", bufs=1))
        pp = ctx.enter_context(tc.tile_pool(name="pp", bufs=3))
        osb = ctx.enter_context(tc.tile_pool(name="osb", bufs=1))
        ffa = ctx.enter_context(tc.tile_pool(name="ffa", bufs=2))
        corr = ctx.enter_context(tc.tile_pool(name="corr", bufs=2))
        stats = ctx.enter_context(tc.tile_pool(name="stats", bufs=5))
        rdp = ctx.enter_context(tc.tile_pool(name="rdp", bufs=2))
        stats_bf = ctx.enter_context(tc.tile_pool(name="stats_bf", bufs=3))
        ps2 = ctx.enter_context(tc.tile_pool(name="ps2", bufs=3, space="PSUM"))
        ps1 = ctx.enter_context(tc.tile_pool(name="ps1", bufs=2, space="PSUM"))

        sq_t = sqpool.tile([128, C, NTOK], BF16)
        pools = {"scr": scr, "sq": sq_t, "stats": stats, "rdp": rdp,
                 "stats_bf": stats_bf, "ps2": ps2, "ps1": ps1}

        ones_t = consts.tile([128, 128], BF16)
        nc.gpsimd.memset(ones_t[:], 1.0)
        eps2_t = consts.tile([128, 1], F32)
        nc.gpsimd.memset(eps2_t[:], EPS)
        tri_t = consts.tile([128, 128], BF16)
        nc.sync.dma_start(tri_t[:], tri_d[:])
        pb_t = consts.tile([128, NB * C], F32)
        nc.sync.dma_start(pb_t[:], pb_d[:])
        fb1_t = consts.tile([128, NB * CF], F32)
        nc.sync.dma_start(fb1_t[:], fb1_d[:])
        fb2_t = consts.tile([128, NB * C], F32)
        nc.sync.dma_start(fb2_t[:], fb2_d[:])
        ow_t = consts.tile([128, C, V], BF16)
        nc.sync.dma_start(ow_t[:], ow_d[:].rearrange("p (c v) -> p c v", v=V))
        ob_t = consts.tile([V, 1], F32)
        nc.sync.dma_start(ob_t[:], ob_d[:])
        csqk_t = consts.tile([1, NB, 2, 128], BF16)
        nc.sync.dma_start(csqk_t[:], csqk_d[:].rearrange(
            "p (l u m) -> p l u m", u=2, m=128))
        csf1_t = consts.tile([1, NB, 4, 128], BF16)
        nc.sync.dma_start(csf1_t[:], csf1_d[:].rearrange(
            "p (l u m) -> p l u m", u=4, m=128))
        csow_t = consts.tile([1, V], BF16)
        nc.sync.dma_start(csow_t[:], csow_d[:])
        lng_t = lnb_t = None
        if ln_general_params:
            lng_t = consts.tile([128, 2 * NB + 1, C], F32)
            nc.sync.dma_start(lng_t[:], lng_d[:].rearrange(
                "p (l c) -> p l c", c=C))
            lnb_t = consts.tile([128, 2 * NB + 1, C], F32)
            nc.sync.dma_start(lnb_t[:], lnb_d[:].rearrange(
                "p (l c) -> p l c", c=C))

        h_t = hpool.tile([128, C, NTOK], BF16)
        nc.sync.dma_start(h_t[:], h0_d[:].rearrange(
            "p (c t) -> p c t", t=NTOK))

        def ln_params(idx):
            if ln_general_params and not ln_trivial[idx]:
                return lng_t[:, idx, :], lnb_t[:, idx, :], False
            return None, None, True

        for i in range(nb_run):
            # ---- load this layer's weights ----
            if i > 0:
                wq_t = wqkv.tile([128, C, 512], BF16, tag="wq")
                nc.sync.dma_start(wq_t[:], wq_d[i].rearrange(
                    "p (c m) -> p c m", m=512))
                wk_t = wqkv.tile([128, C, 512], BF16, tag="wk")
                nc.sync.dma_start(wk_t[:], wk_d[i].rearrange(
                    "p (c m) -> p c m", m=512))
                wv_t = wqkv.tile([128, C, 512], BF16, tag="wv")
                nc.sync.dma_start(wv_t[:], wv_d[i].rearrange(
                    "p (c m) -> p c m", m=512))
            pw_t = wqkv.tile([128, C, 512], BF16, tag="pw")
            nc.sync.dma_start(pw_t[:], pw_d[i].rearrange(
                "p (c m) -> p c m", m=512))
            f1_t = wff1.tile([128, C, FF], BF16, tag="f1")
            nc.sync.dma_start(f1_t[:], f1_d[i].rearrange(
                "p (c m) -> p c m", m=FF))
            f2_t = wff2.tile([128, CF, 512], BF16, tag="f2")
            nc.sync.dma_start(f2_t[:], f2_d[i].rearrange(
                "p (c m) -> p c m", m=512))

            # ---- LN1 output for this layer (layer 0: host-computed;
            #      others peeled into the previous layer's FFN emission) ----
            xn = xn_next if i > 0 else None
            qk_corr = corr_en and i > 0 and ln_trivial[2 * i]

            # ---- corrected Q/K for head-pair 0: matmuls run on h_t so the
            #      PE has work while the LN1 chain computes r/z; the drain
            #      applies q = (Wq^T h)*r - z*colsum(Wq). ----
            qk_hp0 = None
            if qk_corr:
                qk_hp0 = qk.tile([128, 2, NTOK], BF16, tag="qk")
                msl0 = slice(0, 128)
                for tt in range(2):
                    r1, z1 = rz1_next[tt]
                    sl = slice(tt * 512, tt * 512 + 512)
                    qkp = ps2.tile([128, 2, 512], F32, tag="ps2")
                    for c in range(C):
                        nc.tensor.matmul(qkp[:, 0, :], wq_t[:, c, msl0],
                                         h_t[:, c, sl],
                                         start=(c == 0), stop=False)
                        nc.tensor.matmul(qkp[:, 1, :], wk_t[:, c, msl0],
                                         h_t[:, c, sl],
                                         start=(c == 0), stop=False)
                    for u in range(2):
                        nc.tensor.matmul(qkp[:, u, :], csqk_t[:, i, u, :],
                                         z1[0:1, :], start=False, stop=True)
                    nc.vector.tensor_tensor(
                        qk_hp0[:, :, sl], qkp[:],
                        r1[:, None, :].to_broadcast((128, 2, 512)),
                        OP.mult)

            # ---- V, token-major: vt[tk, hd*64+d] ----
            vt_t = vt.tile([128, SEQ * NJ, 512], BF16, tag="vt")
            if i == 0:
                nc.sync.dma_start(vt_t[:], v0_d[:].rearrange(
                    "p (g m) -> p g m", m=512))
            else:
                for jp in range(SEQ * NJ // 2):
                    vp = ps2.tile([128, 2, 512], F32, tag="ps2")
                    for u in range(2):
                        jg = jp * 2 + u
                        for c in range(C):
                            nc.tensor.matmul(
                                vp[:, u, :],
                                xn[:, c, jg * 128:(jg + 1) * 128],
                                wv_t[:, c, :],
                                start=(c == 0), stop=(c == C - 1))
                    nc.scalar.copy(vt_t[:, jp * 2:jp * 2 + 2, :], vp[:])

            o_t = osb.tile([128, C, NTOK], BF16, tag="o")

            def emit_den_o(s, hp, p_t):
                base = s * T
                den = ps1.tile([128, 512], F32, tag="ps1")
                for j in range(NJ):
                    off = j * 128
                    njw = T - off
                    for h2 in range(2):
                        nc.tensor.matmul(den[h2 * 64:h2 * 64 + 64, off:512],
                                         ones_t[:, 0:64],
                                         p_t[:, h2, j, 0:njw],
                                         start=(j == 0), stop=(j == NJ - 1),
                                         skip_group_check=True)
                rd = rdp.tile([128, 512], F32, tag="rd")
                nc.vector.reciprocal_approx_fast(out=rd[:], in_=den[:])
                op_ps = ps1.tile([128, 512], F32, tag="ps1")
                for j in range(NJ):
                    off = j * 128
                    njw = T - off
                    for h2 in range(2):
                        head = hp * 2 + h2
                        nc.tensor.matmul(
                            op_ps[h2 * 64:h2 * 64 + 64, off:T],
                            vt_t[:, s * NJ + j, head * 64:head * 64 + 64],
                            p_t[:, h2, j, 0:njw],
                            start=(j == 0), stop=(j == NJ - 1),
                            skip_group_check=True)
                nc.vector.tensor_tensor(o_t[:, hp, base:base + T],
                                        op_ps[:, 0:T], rd[:], OP.mult)

            pending = None
            for hp in range(HP):
                msl = slice(hp * 128, (hp + 1) * 128)
                if hp == 0 and qk_hp0 is not None:
                    qk_t = qk_hp0
                else:
                    qk_t = qk.tile([128, 2, NTOK], BF16, tag="qk")
                    if i == 0:
                        nc.sync.dma_start(
                            qk_t[:],
                            qk0_d[:, hp * 2 * NTOK:(hp + 1) * 2 * NTOK]
                            .rearrange("p (q t) -> p q t", t=NTOK))
                    else:
                        for tt in range(2):
                            sl = slice(tt * 512, tt * 512 + 512)
                            qkp = ps2.tile([128, 2, 512], F32, tag="ps2")
                            for c in range(C):
                                nc.tensor.matmul(qkp[:, 0, :],
                                                 wq_t[:, c, msl],
                                                 xn[:, c, sl],
                                                 start=(c == 0),
                                                 stop=(c == C - 1))
                                nc.tensor.matmul(qkp[:, 1, :],
                                                 wk_t[:, c, msl],
                                                 xn[:, c, sl],
                                                 start=(c == 0),
                                                 stop=(c == C - 1))
                            nc.scalar.copy(qk_t[:, :, sl], qkp[:])

                for s in range(SEQ):
                    base = s * T
                    p_t = pp.tile([128, 2, NJ, 512], BF16, tag="p")
                    for j in range(NJ):
                        off = j * 128
                        njw = T - off
                        sT = ps2.tile([128, 2, 512], F32, tag="ps2")
                        for h2 in range(2):
                            dsl = slice(h2 * 64, h2 * 64 + 64)
                            nc.tensor.matmul(
                                sT[:, h2, 0:njw],
                                qk_t[dsl, 1, base + off:base + off + 128],
                                qk_t[dsl, 0, base + off:base + T],
                                start=True, stop=True)
                        nc.scalar.activation(
                            p_t[:, :, j, 0:njw], sT[:, :, 0:njw],
                            AF.Exp, scale=SCALE)
                        nc.vector.tensor_tensor(
                            p_t[:, :, j, 0:128], p_t[:, :, j, 0:128],
                            tri_t[:, None, :].to_broadcast(
                                (128, 2, 128)), OP.mult)
                    if pending is not None:
                        emit_den_o(*pending)
                    pending = (s, hp, p_t)
            emit_den_o(*pending)

            # ---- attention out projection + residual ----
            for tt in range(2):
                sl = slice(tt * 512, tt * 512 + 512)
                for mcp in range(C // 2):
                    pj = ps2.tile([128, 2, 512], F32, tag="ps2")
                    for u in range(2):
                        mc = mcp * 2 + u
                        for c in range(C):
                            nc.tensor.matmul(
                                pj[:, u, :],
                                pw_t[:, c, mc * 128:(mc + 1) * 128],
                                o_t[:, c, sl],
                                start=(c == 0), stop=(c == C - 1))
                    if pbz:
                        nc.vector.tensor_tensor(
                            h_t[:, mcp * 2:mcp * 2 + 2, sl], pj[:],
                            h_t[:, mcp * 2:mcp * 2 + 2, sl], OP.add)
                    else:
                        for u in range(2):
                            mc = mcp * 2 + u
                            nc.vector.scalar_tensor_tensor(
                                out=h_t[:, mc, sl], in0=pj[:, u, :],
                                scalar=pb_t[:, i * C + mc:i * C + mc + 1],
                                in1=h_t[:, mc, sl], op0=OP.add, op1=OP.add)

            # ---- LN2 + FFN (token-tile split) ----
            g_ap, b_ap, triv = ln_params(2 * i + 1)
            ffn_corr = corr_en and triv
            xn2 = _alloc_xn(pools)
            r2_0, z2_0 = _emit_ln_tt(nc, pools, h_t, ones_t, eps2_t,
                                     g_ap, b_ap, triv, 0, xn2)
            # corrected first FFN1 blocks (tt=0): matmuls on h_t fill the PE
            # while the LN2 chain runs; drain applies r/z + colsum correction.
            corr_fa = []
            if ffn_corr:
                for mfp in range(2):
                    fp = ps2.tile([128, 2, 512], F32, tag="ps2")
                    for u in range(2):
                        mf = mfp * 2 + u
                        for c in range(C):
                            nc.tensor.matmul(
                                fp[:, u, :],
                                f1_t[:, c, mf * 128:(mf + 1) * 128],
                                h_t[:, c, 0:512],
                                start=(c == 0), stop=False)
                    for u in range(2):
                        mf = mfp * 2 + u
                        nc.tensor.matmul(fp[:, u, :], csf1_t[:, i, mf, :],
                                         z2_0[0:1, :], start=False, stop=True)
                    wtmp = corr.tile([128, 2, 512], BF16, tag="corr")
                    nc.vector.tensor_tensor(
                        wtmp[:], fp[:],
                        r2_0[:, None, :].to_broadcast((128, 2, 512)), OP.mult)
                    corr_fa.append(wtmp)
            _emit_ln_tt(nc, pools, h_t, ones_t, eps2_t, g_ap, b_ap, triv,
                        1, xn2)

            for tt in range(2):
                sl = slice(tt * 512, tt * 512 + 512)
                fa = ffa.tile([128, CF, 512], BF16, tag="fa")
                for mfp in range(CF // 2):
                    if tt == 0 and ffn_corr and mfp < 2:
                        src = corr_fa[mfp][:]
                    else:
                        fp = ps2.tile([128, 2, 512], F32, tag="ps2")
                        for u in range(2):
                            mf = mfp * 2 + u
                            for c in range(C):
                                nc.tensor.matmul(
                                    fp[:, u, :],
                                    f1_t[:, c, mf * 128:(mf + 1) * 128],
                                    xn2[:, c, sl],
                                    start=(c == 0), stop=(c == C - 1))
                        src = fp[:]
                    if fb1z:
                        nc.scalar.activation(
                            fa[:, mfp * 2:mfp * 2 + 2, :], src, AF.Relu)
                    else:
                        for u in range(2):
                            mf = mfp * 2 + u
                            nc.scalar.activation(
                                fa[:, mf, :], src[:, u, :], AF.Relu,
                                bias=fb1_t[:, i * CF + mf:i * CF + mf + 1])
                for mcp in range(C // 2):
                    f2p = ps2.tile([128, 2, 512], F32, tag="ps2")
                    for u in range(2):
                        for c16 in range(CF):
                            nc.tensor.matmul(
                                f2p[:, u, :],
                                f2_t[:, c16,
                                     (mcp * 2 + u) * 128:
                                     (mcp * 2 + u + 1) * 128],
                                fa[:, c16, :],
                                start=(c16 == 0), stop=(c16 == CF - 1))
                    if fb2z:
                        nc.vector.tensor_tensor(
                            h_t[:, mcp * 2:mcp * 2 + 2, sl], f2p[:],
                            h_t[:, mcp * 2:mcp * 2 + 2, sl], OP.add)
                    else:
                        for u in range(2):
                            mc = mcp * 2 + u
                            nc.vector.scalar_tensor_tensor(
                                out=h_t[:, mc, sl], in0=f2p[:, u, :],
                                scalar=fb2_t[:, i * C + mc:i * C + mc + 1],
                                in1=h_t[:, mc, sl], op0=OP.add, op1=OP.add)
                # peel next layer's LN1(tt) here so its scalar/vector chain
                # hides behind the other token-tile's FFN matmuls
                if i + 1 < nb_run:
                    if tt == 0:
                        xn_next = _alloc_xn(pools)
                        rz1_next = []
                    g_ap, b_ap, triv = ln_params(2 * (i + 1))
                    rz1_next.append(_emit_ln_tt(nc, pools, h_t, ones_t,
                                                eps2_t, g_ap, b_ap, triv,
                                                tt, xn_next))

        # ---- final LN + logits (corrected: logits matmuls run on h_t) ----
        g_ap, b_ap, triv = (ln_params(2 * NB) if nb_run == NB
                            else (None, None, True))
        lg_sb = consts.tile([V, NTOK], F32)
        if triv and corr_en:
            rzf = [_emit_ln_tt(nc, pools, h_t, ones_t, eps2_t, g_ap, b_ap,
                               triv, tt, None) for tt in range(2)]
            for tt in range(2):
                sl = slice(tt * 512, tt * 512 + 512)
                lg = ps1.tile([V, 512], F32, tag="ps1")
                rf, zf = rzf[tt]
                for c in range(C):
                    nc.tensor.matmul(lg[:], ow_t[:, c, :], h_t[:, c, sl],
                                     start=(c == 0), stop=False)
                nc.tensor.matmul(lg[:], csow_t[:], zf[0:1, :],
                                 start=False, stop=True)
                w16 = rdp.tile([V, 512], F32, tag="rd")
                nc.vector.tensor_tensor(w16[:], lg[:], rf[0:V, :], OP.mult)
                nc.vector.tensor_scalar_add(lg_sb[:, sl], w16[:], ob_t[:])
        else:
            xnf, _ = _emit_ln(nc, pools, h_t, ones_t, eps2_t, g_ap, b_ap,
                              triv)
            for tt in range(2):
                sl = slice(tt * 512, tt * 512 + 512)
                lg = ps1.tile([V, 512], F32, tag="ps1")
                for c in range(C):
                    nc.tensor.matmul(lg[:], ow_t[:, c, :], xnf[:, c, sl],
                                     start=(c == 0), stop=(c == C - 1))
                nc.vector.tensor_scalar_add(lg_sb[:, sl], lg[:], ob_t[:])
        nc.sync.dma_start(out_d[:], lg_sb[:])

    nc.finalize()
    return nc


def prepare_inputs(inputs):
    """Host-side preprocessing: embedding gather, layer-0 LN1+QKV, weight
    layout + bf16 cast.  Returns (shared_map, per_core_maps, flags)."""
    f32 = np.float32
    bf16 = ml_dtypes.bfloat16
    x = np.asarray(inputs["x"]).astype(np.int64)
    emb = np.asarray(inputs["emb"], dtype=f32)
    pos = np.asarray(inputs["pos"], dtype=f32)

    positions = np.minimum(np.arange(T), L - 1)
    h0 = (emb[x] + pos[positions][None, :, :]).astype(bf16).astype(f32)

    # layer-0 LN1 + Q/K/V on host (fp32, then bf16)
    g1 = np.asarray(inputs["ln1_g"][0], dtype=f32)
    b1 = np.asarray(inputs["ln1_b"][0], dtype=f32)
    mu = h0.mean(-1, keepdims=True)
    var = np.square(h0 - mu).mean(-1, keepdims=True)
    xn0 = ((h0 - mu) / np.sqrt(var + EPS) * g1 + b1).astype(bf16).astype(f32)
    wq0 = np.asarray(inputs["wq"][0], dtype=f32).astype(bf16).astype(f32)
    wk0 = np.asarray(inputs["wk"][0], dtype=f32).astype(bf16).astype(f32)
    wv0 = np.asarray(inputs["wv"][0], dtype=f32).astype(bf16).astype(f32)
    # [B, T, NH*HS] with head-major feature order
    q0 = np.einsum('bte,hed->bthd', xn0, wq0).reshape(B, T, NH * HS)
    k0 = np.einsum('bte,hed->bthd', xn0, wk0).reshape(B, T, NH * HS)
    v0 = np.einsum('bte,hed->bthd', xn0, wv0).reshape(B, T, NH * HS)

    def to_dev_lhst(mat, kchunks, mcols):
        m = np.ascontiguousarray(mat.astype(bf16))
        return m.reshape(kchunks, 128, mcols).transpose(1, 0, 2).reshape(
            128, kchunks * mcols)

    wq = np.asarray(inputs["wq"], dtype=f32)
    wk = np.asarray(inputs["wk"], dtype=f32)
    wv = np.asarray(inputs["wv"], dtype=f32)
    pw = np.asarray(inputs["proj_w"], dtype=f32)
    f1 = np.asarray(inputs["ff_w1"], dtype=f32)
    f2 = np.asarray(inputs["ff_w2"], dtype=f32)

    wq_dev = np.stack([to_dev_lhst(wq[i].transpose(1, 0, 2).reshape(E, NH * HS),
                                   C, 512) for i in range(NB)])
    wk_dev = np.stack([to_dev_lhst(wk[i].transpose(1, 0, 2).reshape(E, NH * HS),
                                   C, 512) for i in range(NB)])
    wv_dev = np.stack([to_dev_lhst(wv[i].transpose(1, 0, 2).reshape(E, NH * HS),
                                   C, 512) for i in range(NB)])
    pw_dev = np.stack([to_dev_lhst(pw[i], C, 512) for i in range(NB)])
    f1_dev = np.stack([to_dev_lhst(f1[i], C, FF) for i in range(NB)])
    f2_dev = np.stack([to_dev_lhst(f2[i], CF, 512) for i in range(NB)])

    def vec_dev(v, chunks):
        return np.ascontiguousarray(v.astype(f32).reshape(chunks, 128).T)

    pb = np.asarray(inputs["proj_b"], dtype=f32)
    fb1 = np.asarray(inputs["ff_b1"], dtype=f32)
    fb2 = np.asarray(inputs["ff_b2"], dtype=f32)
    bias_zero = (bool(np.all(pb == 0.0)), bool(np.all(fb1 == 0.0)),
                 bool(np.all(fb2 == 0.0)))
    pb_dev = np.concatenate([vec_dev(pb[i], C) for i in range(NB)], axis=1)
    fb1_dev = np.concatenate([vec_dev(fb1[i], CF) for i in range(NB)], axis=1)
    fb2_dev = np.concatenate([vec_dev(fb2[i], C) for i in range(NB)], axis=1)
    ow_dev = to_dev_lhst(np.asarray(inputs["out_w"], dtype=f32) / TEMP, C, V)
    ob_dev = (np.asarray(inputs["out_b"], dtype=f32) / TEMP).reshape(V, 1)
    tri_dev = np.triu(np.ones((128, 128), dtype=f32)).astype(bf16)

    # negated column sums (of the bf16-cast weights) for corrected blocks;
    # shaped as [1, M] row vectors used as K=1 matmul stationaries.
    def neg_cs(mat, cols):
        mb = mat.astype(bf16).astype(f32)
        return -mb[:, cols].sum(axis=0)

    csqk_dev = np.zeros((1, NB, 2, 128), f32)
    csf1_dev = np.zeros((1, NB, 4, 128), f32)
    for i in range(NB):
        wq_flat = wq[i].transpose(1, 0, 2).reshape(E, NH * HS)
        wk_flat = wk[i].transpose(1, 0, 2).reshape(E, NH * HS)
        csqk_dev[0, i, 0] = neg_cs(wq_flat, slice(0, 128))
        csqk_dev[0, i, 1] = neg_cs(wk_flat, slice(0, 128))
        for mf in range(4):
            csf1_dev[0, i, mf] = neg_cs(f1[i], slice(mf * 128, (mf + 1) * 128))
    csqk_dev = csqk_dev.reshape(1, NB * 2 * 128).astype(bf16)
    csf1_dev = csf1_dev.reshape(1, NB * 4 * 128).astype(bf16)
    csow_dev = np.ascontiguousarray(
        neg_cs(np.asarray(inputs["out_w"], dtype=f32) / TEMP,
               slice(0, V)).reshape(1, V).astype(bf16))

    gs, bs, ln_trivial = [], [], []
    for i in range(NB):
        for nm_g, nm_b in (("ln1_g", "ln1_b"), ("ln2_g", "ln2_b")):
            g = np.asarray(inputs[nm_g][i], dtype=f32)
            b = np.asarray(inputs[nm_b][i], dtype=f32)
            gs.append(vec_dev(g, C))
            bs.append(vec_dev(b, C))
            ln_trivial.append(bool(np.all(g == 1.0) and np.all(b == 0.0)))
    g = np.asarray(inputs["lnf_g"], dtype=f32)
    b = np.asarray(inputs["lnf_b"], dtype=f32)
    gs.append(vec_dev(g, C))
    bs.append(vec_dev(b, C))
    ln_trivial.append(bool(np.all(g == 1.0) and np.all(b == 0.0)))
    lng_dev = np.concatenate(gs, axis=1)
    lnb_dev = np.concatenate(bs, axis=1)

    shared = {
        "wq": wq_dev, "wk": wk_dev, "wv": wv_dev, "pw": pw_dev,
        "f1": f1_dev, "f2": f2_dev, "pb": pb_dev, "fb1": fb1_dev,
        "fb2": fb2_dev, "ow": ow_dev, "ob": ob_dev, "tri": tri_dev,
        "lng": lng_dev, "lnb": lnb_dev, "csqk": csqk_dev, "csf1": csf1_dev,
        "csow": csow_dev,
    }

    per_core = []
    for core in range(NCORES):
        csl = slice(SEQ * core, SEQ * core + SEQ)

        def featmaj(a):                      # [SEQ, T, F] -> [128, F/128*NTOK]
            fT = a[csl].transpose(2, 0, 1).reshape(-1, NTOK)   # [F, NTOK]
            ch = fT.shape[0] // 128
            return np.ascontiguousarray(
                fT.reshape(ch, 128, NTOK).transpose(1, 0, 2).reshape(
                    128, ch * NTOK).astype(bf16))

        h0c = featmaj(h0)                       # [128, C*NTOK]
        # qk0: [128, hp, {q,k}, NTOK]; partition = h2*64+d of the pair
        qf = q0[csl].transpose(2, 0, 1).reshape(NH * HS, NTOK)  # [512, NTOK]
        kf = k0[csl].transpose(2, 0, 1).reshape(NH * HS, NTOK)
        qk0c = np.empty((128, HP, 2, NTOK), dtype=f32)
        for hp in range(HP):
            qk0c[:, hp, 0] = qf[hp * 128:(hp + 1) * 128]
            qk0c[:, hp, 1] = kf[hp * 128:(hp + 1) * 128]
        qk0c = np.ascontiguousarray(
            qk0c.reshape(128, HP * 2 * NTOK).astype(bf16))
        # v0: token-major [128, SEQ*NJ, 512]
        vtok = v0[csl].reshape(NTOK, NH * HS)          # [NTOK, 512]
        v0c = np.ascontiguousarray(
            vtok.reshape(SEQ * NJ, 128, NH * HS).transpose(1, 0, 2).reshape(
                128, SEQ * NJ * 512).astype(bf16))
        per_core.append({"h0": h0c, "qk0": qk0c, "v0": v0c})
    return shared, per_core, (tuple(ln_trivial), bias_zero)


def assemble_output(core_logits):
    """core_logits: list of [V, NTOK] fp32 -> [B, T, V]."""
    out = np.empty((B, T, V), np.float32)
    for core in range(NCORES):
        lg = core_logits[core].reshape(V, SEQ, T)
        out[SEQ * core:SEQ * core + SEQ] = lg.transpose(1, 2, 0)
    return out


def get_program(flags):
    ln_trivial, bias_zero = flags
    key = (ln_trivial, bias_zero)
    if key not in _PROGRAM_CACHE:
        _PROGRAM_CACHE[key] = build_program(list(ln_trivial), bias_zero)
    return _PROGRAM_CACHE[key]


def reset_device():
    """Recover a wedged accelerator (axon session reset). Best-effort."""
    try:
        import ctypes
        import jax
        jax.devices()
        lib = ctypes.CDLL('/opt/axon/libaxon_pjrt.so')
        lib.axon_reset.restype = ctypes.c_int64
        lib.axon_reset()
    except Exception:
        pass


def kernel(**inputs):
    from concourse.bass_utils import run_bass_kernel_spmd
    shared, per_core, flags = prepare_inputs(inputs)
    nc = get_program(flags)
    in_maps = [dict(shared, **per_core[c]) for c in range(NCORES)]
    try:
        res = run_bass_kernel_spmd(nc, in_maps, core_ids=list(range(NCORES)))
    except Exception:
        # A previous (profiled) session can leave the device wedged; reset
        # the axon session and retry once.
        reset_device()
        res = run_bass_kernel_spmd(nc, in_maps, core_ids=list(range(NCORES)))
    return assemble_output([res.results[c]["logits"] for c in range(NCORES)])


# revision 48
# speedup vs baseline: 1.2168x; 1.0175x over previous
"""Trainium2 Bass kernel for nn_AutoregressiveArithmeticTransformer.

6-layer dense transformer: B=16, T=512, E=512, NH=8 heads x HS=64, FF=2048,
V=16, causal attention, pre-LN, learned abacus embedding, logits / 0.8.

Strategy: data-parallel over batch across 8 NeuronCores (2 sequences per
core, no collectives). Activations live feature-major in SBUF
([E-partitions, tokens]); the residual stream is bf16; weights are streamed
per-layer in bf16; all matmuls run in bf16 with fp32 PSUM accumulation.

v2 over the original baseline:
  - layer-0 LN1 + Q/K/V projections precomputed on the host (the embedding
    gather is already host-side); q0/k0/v0 are DMA'd directly.
  - softmax denominator ones-matmuls col-tiled (M=64 pairs at tile
    positions (0,0)/(0,64)) so the two heads of a pair run concurrently
    in the PE array; same for the attention-V matmuls (as before).
  - score pairs land in one 2-bank PSUM tile so exp is ONE activation op
    per key-chunk ([128, 2, njw]); reciprocal and the o*(1/den) multiply
    are one [128,512] op per (seq, head-pair).
  - q/k PSUM results share a 2-bank tile -> single cast per (hp, tt);
    V-projection copies merged in pairs; FFN1 relu merged in pairs.
  - bf16 residual stream: no fp32->bf16 cast before LN stats matmuls.
  - LN apply multiplies and causal tri-mask multiplies run on the
    otherwise-idle GpSimd engine.
"""

import numpy as np
import ml_dtypes

import concourse.bacc as bacc
import concourse.tile as tile
from concourse import mybir

F32 = mybir.dt.float32
BF16 = mybir.dt.bfloat16
AF = mybir.ActivationFunctionType
OP = mybir.AluOpType

# Model constants (hardcoded per contest contract)
V, E, NH, HS, FF, NB, L = 16, 512, 8, 64, 2048, 6, 512
B, T = 16, 512
TEMP = 1.0 * 0.8
EPS = 1e-5
SCALE = HS ** -0.5  # 0.125

NCORES = 8
SEQ = 2              # sequences per core
NTOK = SEQ * T       # 1024 tokens per core
C = E // 128         # 4 E-chunks
CF = FF // 128       # 16 FF-chunks
HP = NH // 2         # 4 head-pairs
NJ = T // 128        # 4 tk chunks per sequence

_PROGRAM_CACHE = {}


def _emit_ln_tt(nc, pools, h_t, ones_t, eps2_t, g_ap, b_ap, trivial, tt, xn):
    """One token-tile of LayerNorm from bf16 h_t into caller-alloc'd xn.

    Chain is kept DVE-local with a single ACT hop (Sqrt):
      mu = s1/E; var = s2/E - mu^2; sig = sqrt(var + eps); r = 1/sig;
      xn = (h - mu)*r.
    Returns (r_bf, mu_bf) for LN-corrected consumers.
    """
    stats, stats_bf = pools["stats"], pools["stats_bf"]
    ps1 = pools["ps1"]
    sq = pools["sq"]
    sl = slice(tt * 512, tt * 512 + 512)
    s1 = ps1.tile([128, 512], F32, tag="ps1")
    s2 = ps1.tile([128, 512], F32, tag="ps1")
    for c in range(C):
        nc.vector.tensor_tensor(sq[:, c, sl], h_t[:, c, sl], h_t[:, c, sl],
                                OP.mult)
        nc.tensor.matmul(s1[:], ones_t[:], h_t[:, c, sl],
                         start=(c == 0), stop=(c == C - 1))
        nc.tensor.matmul(s2[:], ones_t[:], sq[:, c, sl],
                         start=(c == 0), stop=(c == C - 1))
    mu = stats.tile([128, 512], F32, tag="stats")
    nc.vector.tensor_scalar(out=mu[:], in0=s1[:],
                            scalar1=1.0 / float(E), scalar2=None, op0=OP.mult)
    msq = stats.tile([128, 512], F32, tag="stats")
    nc.vector.tensor_tensor(msq[:], mu[:], mu[:], OP.mult)
    var = stats.tile([128, 512], F32, tag="stats")
    nc.vector.scalar_tensor_tensor(out=var[:], in0=s2[:],
                                   scalar=1.0 / float(E), in1=msq[:],
                                   op0=OP.mult, op1=OP.subtract)
    std = stats.tile([128, 512], F32, tag="stats")
    nc.scalar.activation(std[:], var[:], AF.Sqrt, bias=eps2_t[:])
    rc = stats.tile([128, 512], F32, tag="stats")
    nc.vector.reciprocal_approx_fast(out=rc[:], in_=std[:])
    r_bf = stats_bf.tile([128, 512], BF16, tag="r_bf")
    nc.scalar.copy(r_bf[:], rc[:])
    mu_bf = stats_bf.tile([128, 512], BF16, tag="mu_bf")
    nc.vector.tensor_scalar(out=mu_bf[:], in0=mu[:], scalar1=1.0,
                            scalar2=None, op0=OP.mult)
    if xn is not None:
        for c in range(C):
            nc.vector.tensor_tensor(xn[:, c, sl], h_t[:, c, sl], mu_bf[:],
                                    OP.subtract)
            nc.vector.tensor_tensor(xn[:, c, sl], xn[:, c, sl], r_bf[:],
                                    OP.mult)
            if not trivial:
                nc.vector.tensor_scalar(out=xn[:, c, sl], in0=xn[:, c, sl],
                                        scalar1=g_ap[:, c:c + 1],
                                        scalar2=b_ap[:, c:c + 1],
                                        op0=OP.mult, op1=OP.add)
    return r_bf, mu_bf


def _alloc_xn(pools):
    return pools["scr"].tile([128, C, NTOK], BF16, tag="scratch", name="xnt")


def _emit_ln(nc, pools, h_t, ones_t, eps2_t, g_ap, b_ap, trivial):
    xn = _alloc_xn(pools)
    rz = []
    for tt in range(2):
        rz.append(_emit_ln_tt(nc, pools, h_t, ones_t, eps2_t, g_ap, b_ap,
                              trivial, tt, xn))
    return xn, rz


def build_program(ln_trivial, bias_zero, nb_run=NB, ln_general_params=True):
    """Build the Bass program.

    ln_trivial: list of NB*2+1 bools (ln1/ln2 per layer then lnf); when True
    the g/b application op is skipped.  bias_zero: (pb, fb1, fb2) all-zero
    flags enabling merged residual/relu fast paths."""
    import os
    corr_en = os.environ.get("KERNEL_CORR", "1") == "1"
    pbz, fb1z, fb2z = bias_zero
    nc = bacc.Bacc(None, target_bir_lowering=False)

    h0_d = nc.dram_tensor("h0", [128, C * NTOK], BF16, kind="ExternalInput")
    qk0_d = nc.dram_tensor("qk0", [128, HP * 2 * NTOK], BF16,
                           kind="ExternalInput")
    v0_d = nc.dram_tensor("v0", [128, SEQ * NJ * 512], BF16,
                          kind="ExternalInput")
    wq_d = nc.dram_tensor("wq", [NB, 128, C * 512], BF16, kind="ExternalInput")
    wk_d = nc.dram_tensor("wk", [NB, 128, C * 512], BF16, kind="ExternalInput")
    wv_d = nc.dram_tensor("wv", [NB, 128, C * 512], BF16, kind="ExternalInput")
    pw_d = nc.dram_tensor("pw", [NB, 128, C * 512], BF16, kind="ExternalInput")
    f1_d = nc.dram_tensor("f1", [NB, 128, C * FF], BF16, kind="ExternalInput")
    f2_d = nc.dram_tensor("f2", [NB, 128, CF * 512], BF16, kind="ExternalInput")
    pb_d = nc.dram_tensor("pb", [128, NB * C], F32, kind="ExternalInput")
    fb1_d = nc.dram_tensor("fb1", [128, NB * CF], F32, kind="ExternalInput")
    fb2_d = nc.dram_tensor("fb2", [128, NB * C], F32, kind="ExternalInput")
    ow_d = nc.dram_tensor("ow", [128, C * V], BF16, kind="ExternalInput")
    ob_d = nc.dram_tensor("ob", [V, 1], F32, kind="ExternalInput")
    tri_d = nc.dram_tensor("tri", [128, 128], BF16, kind="ExternalInput")
    # negated column sums for LN-corrected first blocks (row vectors: these
    # are K=1 matmul stationaries accumulating cs (x) z into PSUM)
    csqk_d = nc.dram_tensor("csqk", [128, NB * 2 * 128], BF16,
                            kind="ExternalInput")
    csf1_d = nc.dram_tensor("csf1", [1, NB * 4 * 128], BF16,
                            kind="ExternalInput")
    csow_d = nc.dram_tensor("csow", [1, V], BF16, kind="ExternalInput")
    lng_d = lnb_d = None
    if ln_general_params:
        lng_d = nc.dram_tensor("lng", [128, (2 * NB + 1) * C], F32,
                               kind="ExternalInput")
        lnb_d = nc.dram_tensor("lnb", [128, (2 * NB + 1) * C], F32,
                               kind="ExternalInput")
    out_d = nc.dram_tensor("logits", [V, NTOK], F32, kind="ExternalOutput")

    from contextlib import ExitStack
    with ExitStack() as ctx:
        tc = ctx.enter_context(tile.TileContext(nc))
        consts = ctx.enter_context(tc.tile_pool(name="consts", bufs=1))
        hpool = ctx.enter_context(tc.tile_pool(name="hpool", bufs=1))
        wqkv = ctx.enter_context(tc.tile_pool(name="wqkv", bufs=1))
        wff1 = ctx.enter_context(tc.tile_pool(name="wff1", bufs=1))
        wff2 = ctx.enter_context(tc.tile_pool(name="wff2", bufs=1))
        scr = ctx.enter_context(tc.tile_pool(name="scr", bufs=3))
        sqpool = ctx.enter_context(tc.tile_pool(name="sqp", bufs=1))
        qk = ctx.enter_context(tc.tile_pool(name="qk", bufs=2))
        vt = ctx.enter_context(tc.tile_pool(name="vt", bufs=1))
        pp = ctx.enter_context(tc.tile_pool(name="pp", bufs=3))
        osb = ctx.enter_context(tc.tile_pool(name="osb", bufs=1))
        ffa = ctx.enter_context(tc.tile_pool(name="ffa", bufs=2))
        corr = ctx.enter_context(tc.tile_pool(name="corr", bufs=2))
        stats = ctx.enter_context(tc.tile_pool(name="stats", bufs=5))
        rdp = ctx.enter_context(tc.tile_pool(name="rdp", bufs=2))
        stats_bf = ctx.enter_context(tc.tile_pool(name="stats_bf", bufs=3))
        ps2 = ctx.enter_context(tc.tile_pool(name="ps2", bufs=3, space="PSUM"))
        ps1 = ctx.enter_context(tc.tile_pool(name="ps1", bufs=2, space="PSUM"))

        sq_t = sqpool.tile([128, C, NTOK], BF16)
        pools = {"scr": scr, "sq": sq_t, "stats": stats, "rdp": rdp,
                 "stats_bf": stats_bf, "ps2": ps2, "ps1": ps1}

        ones_t = consts.tile([128, 128], BF16)
        nc.gpsimd.memset(ones_t[:], 1.0)
        eps2_t = consts.tile([128, 1], F32)
        nc.gpsimd.memset(eps2_t[:], EPS)

        # startup-critical DMAs first: layer-0 attention consumes q/k/v
        # immediately; everything else can stream in behind them.
        qk0_pre = {}
        for hp in (0, 1):
            t = qk.tile([128, 2, NTOK], BF16, tag="qk")
            nc.sync.dma_start(
                t[:], qk0_d[:, hp * 2 * NTOK:(hp + 1) * 2 * NTOK]
                .rearrange("p (q t) -> p q t", t=NTOK))
            qk0_pre[hp] = t
        vt0_pre = vt.tile([128, SEQ * NJ, 512], BF16, tag="vt")
        nc.sync.dma_start(vt0_pre[:], v0_d[:].rearrange(
            "p (g m) -> p g m", m=512))
        tri_t = consts.tile([128, 128], BF16)
        nc.sync.dma_start(tri_t[:], tri_d[:])
        h_t = hpool.tile([128, C, NTOK], BF16)
        nc.sync.dma_start(h_t[:], h0_d[:].rearrange(
            "p (c t) -> p c t", t=NTOK))
        pb_t = consts.tile([128, NB * C], F32)
        nc.sync.dma_start(pb_t[:], pb_d[:])
        fb1_t = consts.tile([128, NB * CF], F32)
        nc.sync.dma_start(fb1_t[:], fb1_d[:])
        fb2_t = consts.tile([128, NB * C], F32)
        nc.sync.dma_start(fb2_t[:], fb2_d[:])
        ow_t = consts.tile([128, C, V], BF16)
        nc.sync.dma_start(ow_t[:], ow_d[:].rearrange("p (c v) -> p c v", v=V))
        ob_t = consts.tile([V, 1], F32)
        nc.sync.dma_start(ob_t[:], ob_d[:])
        # rows 0 and 32 hold hp0/hp1 colsums (legal K=1 tile positions)
        csqk_t = consts.tile([128, NB, 2, 128], BF16)
        nc.sync.dma_start(csqk_t[:], csqk_d[:].rearrange(
            "p (l u m) -> p l u m", u=2, m=128))
        csf1_t = consts.tile([1, NB, 4, 128], BF16)
        nc.sync.dma_start(csf1_t[:], csf1_d[:].rearrange(
            "p (l u m) -> p l u m", u=4, m=128))
        csow_t = consts.tile([1, V], BF16)
        nc.sync.dma_start(csow_t[:], csow_d[:])
        lng_t = lnb_t = None
        if ln_general_params:
            lng_t = consts.tile([128, 2 * NB + 1, C], F32)
            nc.sync.dma_start(lng_t[:], lng_d[:].rearrange(
                "p (l c) -> p l c", c=C))
            lnb_t = consts.tile([128, 2 * NB + 1, C], F32)
            nc.sync.dma_start(lnb_t[:], lnb_d[:].rearrange(
                "p (l c) -> p l c", c=C))

        def ln_params(idx):
            if ln_general_params and not ln_trivial[idx]:
                return lng_t[:, idx, :], lnb_t[:, idx, :], False
            return None, None, True

        lg_sb = consts.tile([V, NTOK], F32)
        final_peeled = False
        for i in range(nb_run):
            # ---- load this layer's weights ----
            if i > 0:
                wq_t = wqkv.tile([128, C, 512], BF16, tag="wq")
                nc.sync.dma_start(wq_t[:], wq_d[i].rearrange(
                    "p (c m) -> p c m", m=512))
                wk_t = wqkv.tile([128, C, 512], BF16, tag="wk")
                nc.sync.dma_start(wk_t[:], wk_d[i].rearrange(
                    "p (c m) -> p c m", m=512))
                wv_t = wqkv.tile([128, C, 512], BF16, tag="wv")
                nc.sync.dma_start(wv_t[:], wv_d[i].rearrange(
                    "p (c m) -> p c m", m=512))
            pw_t = wqkv.tile([128, C, 512], BF16, tag="pw")
            nc.sync.dma_start(pw_t[:], pw_d[i].rearrange(
                "p (c m) -> p c m", m=512))
            f1_t = wff1.tile([128, C, FF], BF16, tag="f1")
            nc.sync.dma_start(f1_t[:], f1_d[i].rearrange(
                "p (c m) -> p c m", m=FF))
            f2_t = wff2.tile([128, CF, 512], BF16, tag="f2")
            nc.sync.dma_start(f2_t[:], f2_d[i].rearrange(
                "p (c m) -> p c m", m=512))

            # ---- LN1 output for this layer (layer 0: host-computed;
            #      others peeled into the previous layer's FFN emission) ----
            xn = xn_next if i > 0 else None
            qk_corr = corr_en and i > 0 and ln_trivial[2 * i]

            # ---- corrected Q/K for head-pairs 0,1: matmuls run on h_t so
            #      the PE has work while the LN1 chain computes r/mu; a K=1
            #      matmul accumulates colsum (x) mu, drain multiplies by r:
            #      q = r*((Wq^T h) - mu*colsum(Wq)) = Wq^T xn exactly. ----
            qk_pre = {}
            if qk_corr:
                for hpc in (0, 1):
                    row = 32 * hpc
                    qk_c = qk.tile([128, 2, NTOK], BF16, tag="qk")
                    mslc = slice(hpc * 128, (hpc + 1) * 128)
                    for tt in range(2):
                        r1, mu1 = rz1_next[tt]
                        sl = slice(tt * 512, tt * 512 + 512)
                        qkp = ps2.tile([128, 2, 512], F32, tag="ps2")
                        for c in range(C):
                            nc.tensor.matmul(qkp[:, 0, :], wq_t[:, c, mslc],
                                             h_t[:, c, sl],
                                             start=(c == 0), stop=False)
                            nc.tensor.matmul(qkp[:, 1, :], wk_t[:, c, mslc],
                                             h_t[:, c, sl],
                                             start=(c == 0), stop=False)
                        for u in range(2):
                            nc.tensor.matmul(
                                qkp[:, u, :],
                                csqk_t[row:row + 1, i, u, :],
                                mu1[row:row + 1, :], start=False, stop=True)
                        nc.vector.tensor_tensor(
                            qk_c[:, :, sl], qkp[:],
                            r1[:, None, :].to_broadcast((128, 2, 512)),
                            OP.mult)
                    qk_pre[hpc] = qk_c
            elif i == 0:
                qk_pre = {0: qk0_pre[0], 1: qk0_pre[1]}

            # ---- V, token-major: vt[tk, hd*64+d] ----
            if i == 0:
                vt_t = vt0_pre
            else:
                vt_t = vt.tile([128, SEQ * NJ, 512], BF16, tag="vt")
                for jp in range(SEQ * NJ // 2):
                    vp = ps2.tile([128, 2, 512], F32, tag="ps2")
                    for u in range(2):
                        jg = jp * 2 + u
                        for c in range(C):
                            nc.tensor.matmul(
                                vp[:, u, :],
                                xn[:, c, jg * 128:(jg + 1) * 128],
                                wv_t[:, c, :],
                                start=(c == 0), stop=(c == C - 1))
                    nc.scalar.copy(vt_t[:, jp * 2:jp * 2 + 2, :], vp[:])

            o_t = osb.tile([128, C, NTOK], BF16, tag="o")

            def emit_den_o(s, hp, p_t):
                base = s * T
                den = ps1.tile([128, 512], F32, tag="ps1")
                for j in range(NJ):
                    off = j * 128
                    njw = T - off
                    for h2 in range(2):
                        nc.tensor.matmul(den[h2 * 64:h2 * 64 + 64, off:512],
                                         ones_t[:, 0:64],
                                         p_t[:, h2, j, 0:njw],
                                         start=(j == 0), stop=(j == NJ - 1),
                                         skip_group_check=True)
                rd = rdp.tile([128, 512], F32, tag="rd")
                nc.vector.reciprocal_approx_fast(out=rd[:], in_=den[:])
                op_ps = ps1.tile([128, 512], F32, tag="ps1")
                for j in range(NJ):
                    off = j * 128
                    njw = T - off
                    for h2 in range(2):
                        head = hp * 2 + h2
                        nc.tensor.matmul(
                            op_ps[h2 * 64:h2 * 64 + 64, off:T],
                            vt_t[:, s * NJ + j, head * 64:head * 64 + 64],
                            p_t[:, h2, j, 0:njw],
                            start=(j == 0), stop=(j == NJ - 1),
                            skip_group_check=True)
                nc.vector.tensor_tensor(o_t[:, hp, base:base + T],
                                        op_ps[:, 0:T], rd[:], OP.mult)

            pending = None
            for hp in range(HP):
                msl = slice(hp * 128, (hp + 1) * 128)
                if hp in qk_pre:
                    qk_t = qk_pre[hp]
                else:
                    qk_t = qk.tile([128, 2, NTOK], BF16, tag="qk")
                    if i == 0:
                        nc.sync.dma_start(
                            qk_t[:],
                            qk0_d[:, hp * 2 * NTOK:(hp + 1) * 2 * NTOK]
                            .rearrange("p (q t) -> p q t", t=NTOK))
                    else:
                        for tt in range(2):
                            sl = slice(tt * 512, tt * 512 + 512)
                            qkp = ps2.tile([128, 2, 512], F32, tag="ps2")
                            for c in range(C):
                                nc.tensor.matmul(qkp[:, 0, :],
                                                 wq_t[:, c, msl],
                                                 xn[:, c, sl],
                                                 start=(c == 0),
                                                 stop=(c == C - 1))
                                nc.tensor.matmul(qkp[:, 1, :],
                                                 wk_t[:, c, msl],
                                                 xn[:, c, sl],
                                                 start=(c == 0),
                                                 stop=(c == C - 1))
                            nc.scalar.copy(qk_t[:, :, sl], qkp[:])

                for s in range(SEQ):
                    base = s * T
                    p_t = pp.tile([128, 2, NJ, 512], BF16, tag="p")
                    for j in range(NJ):
                        off = j * 128
                        njw = T - off
                        sT = ps2.tile([128, 2, 512], F32, tag="ps2")
                        for h2 in range(2):
                            dsl = slice(h2 * 64, h2 * 64 + 64)
                            nc.tensor.matmul(
                                sT[:, h2, 0:njw],
                                qk_t[dsl, 1, base + off:base + off + 128],
                                qk_t[dsl, 0, base + off:base + T],
                                start=True, stop=True)
                        nc.scalar.activation(
                            p_t[:, :, j, 0:njw], sT[:, :, 0:njw],
                            AF.Exp, scale=SCALE)
                        nc.vector.tensor_tensor(
                            p_t[:, :, j, 0:128], p_t[:, :, j, 0:128],
                            tri_t[:, None, :].to_broadcast(
                                (128, 2, 128)), OP.mult)
                    if pending is not None:
                        emit_den_o(*pending)
                    pending = (s, hp, p_t)
            emit_den_o(*pending)

            # ---- attention out projection + residual ----
            for tt in range(2):
                sl = slice(tt * 512, tt * 512 + 512)
                for mcp in range(C // 2):
                    pj = ps2.tile([128, 2, 512], F32, tag="ps2")
                    for u in range(2):
                        mc = mcp * 2 + u
                        for c in range(C):
                            nc.tensor.matmul(
                                pj[:, u, :],
                                pw_t[:, c, mc * 128:(mc + 1) * 128],
                                o_t[:, c, sl],
                                start=(c == 0), stop=(c == C - 1))
                    if pbz:
                        nc.vector.tensor_tensor(
                            h_t[:, mcp * 2:mcp * 2 + 2, sl], pj[:],
                            h_t[:, mcp * 2:mcp * 2 + 2, sl], OP.add)
                    else:
                        for u in range(2):
                            mc = mcp * 2 + u
                            nc.vector.scalar_tensor_tensor(
                                out=h_t[:, mc, sl], in0=pj[:, u, :],
                                scalar=pb_t[:, i * C + mc:i * C + mc + 1],
                                in1=h_t[:, mc, sl], op0=OP.add, op1=OP.add)

            # ---- LN2 + FFN (token-tile split) ----
            g_ap, b_ap, triv = ln_params(2 * i + 1)
            ffn_corr = corr_en and triv
            xn2 = _alloc_xn(pools)
            r2_0, z2_0 = _emit_ln_tt(nc, pools, h_t, ones_t, eps2_t,
                                     g_ap, b_ap, triv, 0, xn2)
            # corrected first FFN1 blocks (tt=0): matmuls on h_t fill the PE
            # while the LN2 chain runs; drain applies r/z + colsum correction.
            corr_fa = []
            if ffn_corr:
                for mfp in range(2):
                    fp = ps2.tile([128, 2, 512], F32, tag="ps2")
                    for u in range(2):
                        mf = mfp * 2 + u
                        for c in range(C):
                            nc.tensor.matmul(
                                fp[:, u, :],
                                f1_t[:, c, mf * 128:(mf + 1) * 128],
                                h_t[:, c, 0:512],
                                start=(c == 0), stop=False)
                    for u in range(2):
                        mf = mfp * 2 + u
                        nc.tensor.matmul(fp[:, u, :], csf1_t[:, i, mf, :],
                                         z2_0[0:1, :], start=False, stop=True)
                    wtmp = corr.tile([128, 2, 512], BF16, tag="corr")
                    nc.vector.tensor_tensor(
                        wtmp[:], fp[:],
                        r2_0[:, None, :].to_broadcast((128, 2, 512)), OP.mult)
                    corr_fa.append(wtmp)
            _emit_ln_tt(nc, pools, h_t, ones_t, eps2_t, g_ap, b_ap, triv,
                        1, xn2)

            for tt in range(2):
                sl = slice(tt * 512, tt * 512 + 512)
                fa = ffa.tile([128, CF, 512], BF16, tag="fa")
                for mfp in range(CF // 2):
                    if tt == 0 and ffn_corr and mfp < 2:
                        src = corr_fa[mfp][:]
                    else:
                        fp = ps2.tile([128, 2, 512], F32, tag="ps2")
                        for u in range(2):
                            mf = mfp * 2 + u
                            for c in range(C):
                                nc.tensor.matmul(
                                    fp[:, u, :],
                                    f1_t[:, c, mf * 128:(mf + 1) * 128],
                                    xn2[:, c, sl],
                                    start=(c == 0), stop=(c == C - 1))
                        src = fp[:]
                    if fb1z:
                        nc.scalar.activation(
                            fa[:, mfp * 2:mfp * 2 + 2, :], src, AF.Relu)
                    else:
                        for u in range(2):
                            mf = mfp * 2 + u
                            nc.scalar.activation(
                                fa[:, mf, :], src[:, u, :], AF.Relu,
                                bias=fb1_t[:, i * CF + mf:i * CF + mf + 1])
                for mcp in range(C // 2):
                    f2p = ps2.tile([128, 2, 512], F32, tag="ps2")
                    for u in range(2):
                        for c16 in range(CF):
                            nc.tensor.matmul(
                                f2p[:, u, :],
                                f2_t[:, c16,
                                     (mcp * 2 + u) * 128:
                                     (mcp * 2 + u + 1) * 128],
                                fa[:, c16, :],
                                start=(c16 == 0), stop=(c16 == CF - 1))
                    if fb2z:
                        nc.vector.tensor_tensor(
                            h_t[:, mcp * 2:mcp * 2 + 2, sl], f2p[:],
                            h_t[:, mcp * 2:mcp * 2 + 2, sl], OP.add)
                    else:
                        for u in range(2):
                            mc = mcp * 2 + u
                            nc.vector.scalar_tensor_tensor(
                                out=h_t[:, mc, sl], in0=f2p[:, u, :],
                                scalar=fb2_t[:, i * C + mc:i * C + mc + 1],
                                in1=h_t[:, mc, sl], op0=OP.add, op1=OP.add)
                # peel next layer's LN1(tt) here so its scalar/vector chain
                # hides behind the other token-tile's FFN matmuls; on the
                # last layer peel the final LN + corrected logits instead
                if i + 1 < nb_run:
                    if tt == 0:
                        xn_next = _alloc_xn(pools)
                        rz1_next = []
                    g_ap, b_ap, triv = ln_params(2 * (i + 1))
                    rz1_next.append(_emit_ln_tt(nc, pools, h_t, ones_t,
                                                eps2_t, g_ap, b_ap, triv,
                                                tt, xn_next))
                else:
                    gf, bf_, trivf = (ln_params(2 * NB) if nb_run == NB
                                      else (None, None, True))
                    if trivf and corr_en:
                        rf, muf = _emit_ln_tt(nc, pools, h_t, ones_t, eps2_t,
                                              gf, bf_, trivf, tt, None)
                        lg = ps1.tile([V, 512], F32, tag="ps1")
                        for c in range(C):
                            nc.tensor.matmul(lg[:], ow_t[:, c, :],
                                             h_t[:, c, sl],
                                             start=(c == 0), stop=False)
                        nc.tensor.matmul(lg[:], csow_t[:], muf[0:1, :],
                                         start=False, stop=True)
                        w16 = rdp.tile([V, 512], F32, tag="rd")
                        nc.vector.tensor_tensor(w16[:], lg[:], rf[0:V, :],
                                                OP.mult)
                        nc.vector.tensor_scalar_add(lg_sb[:, sl], w16[:],
                                                    ob_t[:])
                        final_peeled = True

        # ---- final LN + logits (fallback when not peeled above) ----
        g_ap, b_ap, triv = (ln_params(2 * NB) if nb_run == NB
                            else (None, None, True))
        if not final_peeled:
            xnf, _ = _emit_ln(nc, pools, h_t, ones_t, eps2_t, g_ap, b_ap,
                              triv)
            for tt in range(2):
                sl = slice(tt * 512, tt * 512 + 512)
                lg = ps1.tile([V, 512], F32, tag="ps1")
                for c in range(C):
                    nc.tensor.matmul(lg[:], ow_t[:, c, :], xnf[:, c, sl],
                                     start=(c == 0), stop=(c == C - 1))
                nc.vector.tensor_scalar_add(lg_sb[:, sl], lg[:], ob_t[:])
        nc.sync.dma_start(out_d[:], lg_sb[:])

    nc.finalize()
    return nc


def prepare_inputs(inputs):
    """Host-side preprocessing: embedding gather, layer-0 LN1+QKV, weight
    layout + bf16 cast.  Returns (shared_map, per_core_maps, flags)."""
    f32 = np.float32
    bf16 = ml_dtypes.bfloat16
    x = np.asarray(inputs["x"]).astype(np.int64)
    emb = np.asarray(inputs["emb"], dtype=f32)
    pos = np.asarray(inputs["pos"], dtype=f32)

    positions = np.minimum(np.arange(T), L - 1)
    h0 = (emb[x] + pos[positions][None, :, :]).astype(bf16).astype(f32)

    # layer-0 LN1 + Q/K/V on host (fp32, then bf16)
    g1 = np.asarray(inputs["ln1_g"][0], dtype=f32)
    b1 = np.asarray(inputs["ln1_b"][0], dtype=f32)
    mu = h0.mean(-1, keepdims=True)
    var = np.square(h0 - mu).mean(-1, keepdims=True)
    xn0 = ((h0 - mu) / np.sqrt(var + EPS) * g1 + b1).astype(bf16).astype(f32)
    wq0 = np.asarray(inputs["wq"][0], dtype=f32).astype(bf16).astype(f32)
    wk0 = np.asarray(inputs["wk"][0], dtype=f32).astype(bf16).astype(f32)
    wv0 = np.asarray(inputs["wv"][0], dtype=f32).astype(bf16).astype(f32)
    # [B, T, NH*HS] with head-major feature order
    q0 = np.einsum('bte,hed->bthd', xn0, wq0).reshape(B, T, NH * HS)
    k0 = np.einsum('bte,hed->bthd', xn0, wk0).reshape(B, T, NH * HS)
    v0 = np.einsum('bte,hed->bthd', xn0, wv0).reshape(B, T, NH * HS)

    def to_dev_lhst(mat, kchunks, mcols):
        m = np.ascontiguousarray(mat.astype(bf16))
        return m.reshape(kchunks, 128, mcols).transpose(1, 0, 2).reshape(
            128, kchunks * mcols)

    wq = np.asarray(inputs["wq"], dtype=f32)
    wk = np.asarray(inputs["wk"], dtype=f32)
    wv = np.asarray(inputs["wv"], dtype=f32)
    pw = np.asarray(inputs["proj_w"], dtype=f32)
    f1 = np.asarray(inputs["ff_w1"], dtype=f32)
    f2 = np.asarray(inputs["ff_w2"], dtype=f32)

    wq_dev = np.stack([to_dev_lhst(wq[i].transpose(1, 0, 2).reshape(E, NH * HS),
                                   C, 512) for i in range(NB)])
    wk_dev = np.stack([to_dev_lhst(wk[i].transpose(1, 0, 2).reshape(E, NH * HS),
                                   C, 512) for i in range(NB)])
    wv_dev = np.stack([to_dev_lhst(wv[i].transpose(1, 0, 2).reshape(E, NH * HS),
                                   C, 512) for i in range(NB)])
    pw_dev = np.stack([to_dev_lhst(pw[i], C, 512) for i in range(NB)])
    f1_dev = np.stack([to_dev_lhst(f1[i], C, FF) for i in range(NB)])
    f2_dev = np.stack([to_dev_lhst(f2[i], CF, 512) for i in range(NB)])

    def vec_dev(v, chunks):
        return np.ascontiguousarray(v.astype(f32).reshape(chunks, 128).T)

    pb = np.asarray(inputs["proj_b"], dtype=f32)
    fb1 = np.asarray(inputs["ff_b1"], dtype=f32)
    fb2 = np.asarray(inputs["ff_b2"], dtype=f32)
    bias_zero = (bool(np.all(pb == 0.0)), bool(np.all(fb1 == 0.0)),
                 bool(np.all(fb2 == 0.0)))
    pb_dev = np.concatenate([vec_dev(pb[i], C) for i in range(NB)], axis=1)
    fb1_dev = np.concatenate([vec_dev(fb1[i], CF) for i in range(NB)], axis=1)
    fb2_dev = np.concatenate([vec_dev(fb2[i], C) for i in range(NB)], axis=1)
    ow_dev = to_dev_lhst(np.asarray(inputs["out_w"], dtype=f32) / TEMP, C, V)
    ob_dev = (np.asarray(inputs["out_b"], dtype=f32) / TEMP).reshape(V, 1)
    tri_dev = np.triu(np.ones((128, 128), dtype=f32)).astype(bf16)

    # negated column sums (of the bf16-cast weights) for corrected blocks;
    # shaped as [1, M] row vectors used as K=1 matmul stationaries.
    def neg_cs(mat, cols):
        mb = mat.astype(bf16).astype(f32)
        return -mb[:, cols].sum(axis=0)

    csqk_dev = np.zeros((128, NB, 2, 128), f32)
    csf1_dev = np.zeros((1, NB, 4, 128), f32)
    for i in range(NB):
        wq_flat = wq[i].transpose(1, 0, 2).reshape(E, NH * HS)
        wk_flat = wk[i].transpose(1, 0, 2).reshape(E, NH * HS)
        for hpc in (0, 1):
            cols = slice(hpc * 128, (hpc + 1) * 128)
            csqk_dev[32 * hpc, i, 0] = neg_cs(wq_flat, cols)
            csqk_dev[32 * hpc, i, 1] = neg_cs(wk_flat, cols)
        for mf in range(4):
            csf1_dev[0, i, mf] = neg_cs(f1[i], slice(mf * 128, (mf + 1) * 128))
    csqk_dev = csqk_dev.reshape(128, NB * 2 * 128).astype(bf16)
    csf1_dev = csf1_dev.reshape(1, NB * 4 * 128).astype(bf16)
    csow_dev = np.ascontiguousarray(
        neg_cs(np.asarray(inputs["out_w"], dtype=f32) / TEMP,
               slice(0, V)).reshape(1, V).astype(bf16))

    gs, bs, ln_trivial = [], [], []
    for i in range(NB):
        for nm_g, nm_b in (("ln1_g", "ln1_b"), ("ln2_g", "ln2_b")):
            g = np.asarray(inputs[nm_g][i], dtype=f32)
            b = np.asarray(inputs[nm_b][i], dtype=f32)
            gs.append(vec_dev(g, C))
            bs.append(vec_dev(b, C))
            ln_trivial.append(bool(np.all(g == 1.0) and np.all(b == 0.0)))
    g = np.asarray(inputs["lnf_g"], dtype=f32)
    b = np.asarray(inputs["lnf_b"], dtype=f32)
    gs.append(vec_dev(g, C))
    bs.append(vec_dev(b, C))
    ln_trivial.append(bool(np.all(g == 1.0) and np.all(b == 0.0)))
    lng_dev = np.concatenate(gs, axis=1)
    lnb_dev = np.concatenate(bs, axis=1)

    shared = {
        "wq": wq_dev, "wk": wk_dev, "wv": wv_dev, "pw": pw_dev,
        "f1": f1_dev, "f2": f2_dev, "pb": pb_dev, "fb1": fb1_dev,
        "fb2": fb2_dev, "ow": ow_dev, "ob": ob_dev, "tri": tri_dev,
        "lng": lng_dev, "lnb": lnb_dev, "csqk": csqk_dev, "csf1": csf1_dev,
        "csow": csow_dev,
    }

    per_core = []
    for core in range(NCORES):
        csl = slice(SEQ * core, SEQ * core + SEQ)

        def featmaj(a):                      # [SEQ, T, F] -> [128, F/128*NTOK]
            fT = a[csl].transpose(2, 0, 1).reshape(-1, NTOK)   # [F, NTOK]
            ch = fT.shape[0] // 128
            return np.ascontiguousarray(
                fT.reshape(ch, 128, NTOK).transpose(1, 0, 2).reshape(
                    128, ch * NTOK).astype(bf16))

        h0c = featmaj(h0)                       # [128, C*NTOK]
        # qk0: [128, hp, {q,k}, NTOK]; partition = h2*64+d of the pair
        qf = q0[csl].transpose(2, 0, 1).reshape(NH * HS, NTOK)  # [512, NTOK]
        kf = k0[csl].transpose(2, 0, 1).reshape(NH * HS, NTOK)
        qk0c = np.empty((128, HP, 2, NTOK), dtype=f32)
        for hp in range(HP):
            qk0c[:, hp, 0] = qf[hp * 128:(hp + 1) * 128]
            qk0c[:, hp, 1] = kf[hp * 128:(hp + 1) * 128]
        qk0c = np.ascontiguousarray(
            qk0c.reshape(128, HP * 2 * NTOK).astype(bf16))
        # v0: token-major [128, SEQ*NJ, 512]
        vtok = v0[csl].reshape(NTOK, NH * HS)          # [NTOK, 512]
        v0c = np.ascontiguousarray(
            vtok.reshape(SEQ * NJ, 128, NH * HS).transpose(1, 0, 2).reshape(
                128, SEQ * NJ * 512).astype(bf16))
        per_core.append({"h0": h0c, "qk0": qk0c, "v0": v0c})
    return shared, per_core, (tuple(ln_trivial), bias_zero)


def assemble_output(core_logits):
    """core_logits: list of [V, NTOK] fp32 -> [B, T, V]."""
    out = np.empty((B, T, V), np.float32)
    for core in range(NCORES):
        lg = core_logits[core].reshape(V, SEQ, T)
        out[SEQ * core:SEQ * core + SEQ] = lg.transpose(1, 2, 0)
    return out


def get_program(flags):
    ln_trivial, bias_zero = flags
    key = (ln_trivial, bias_zero)
    if key not in _PROGRAM_CACHE:
        _PROGRAM_CACHE[key] = build_program(list(ln_trivial), bias_zero)
    return _PROGRAM_CACHE[key]


def reset_device():
    """Recover a wedged accelerator (axon session reset). Best-effort."""
    try:
        import ctypes
        import jax
        jax.devices()
        lib = ctypes.CDLL('/opt/axon/libaxon_pjrt.so')
        lib.axon_reset.restype = ctypes.c_int64
        lib.axon_reset()
    except Exception:
        pass


def kernel(**inputs):
    from concourse.bass_utils import run_bass_kernel_spmd
    shared, per_core, flags = prepare_inputs(inputs)
    nc = get_program(flags)
    in_maps = [dict(shared, **per_core[c]) for c in range(NCORES)]
    try:
        res = run_bass_kernel_spmd(nc, in_maps, core_ids=list(range(NCORES)))
    except Exception:
        # A previous (profiled) session can leave the device wedged; reset
        # the axon session and retry once.
        reset_device()
        res = run_bass_kernel_spmd(nc, in_maps, core_ids=list(range(NCORES)))
    return assemble_output([res.results[c]["logits"] for c in range(NCORES)])


# revision 51
# speedup vs baseline: 1.2313x; 1.0119x over previous
"""Trainium2 Bass kernel for nn_AutoregressiveArithmeticTransformer.

6-layer dense transformer: B=16, T=512, E=512, NH=8 heads x HS=64, FF=2048,
V=16, causal attention, pre-LN, learned abacus embedding, logits / 0.8.

Strategy: data-parallel over batch across 8 NeuronCores (2 sequences per
core, no collectives). Activations live feature-major in SBUF
([E-partitions, tokens]); the residual stream is bf16; weights are streamed
per-layer in bf16; all matmuls run in bf16 with fp32 PSUM accumulation.

v2 over the original baseline:
  - layer-0 LN1 + Q/K/V projections precomputed on the host (the embedding
    gather is already host-side); q0/k0/v0 are DMA'd directly.
  - softmax denominator ones-matmuls col-tiled (M=64 pairs at tile
    positions (0,0)/(0,64)) so the two heads of a pair run concurrently
    in the PE array; same for the attention-V matmuls (as before).
  - score pairs land in one 2-bank PSUM tile so exp is ONE activation op
    per key-chunk ([128, 2, njw]); reciprocal and the o*(1/den) multiply
    are one [128,512] op per (seq, head-pair).
  - q/k PSUM results share a 2-bank tile -> single cast per (hp, tt);
    V-projection copies merged in pairs; FFN1 relu merged in pairs.
  - bf16 residual stream: no fp32->bf16 cast before LN stats matmuls.
  - LN apply multiplies and causal tri-mask multiplies run on the
    otherwise-idle GpSimd engine.
"""

import numpy as np
import ml_dtypes

import concourse.bacc as bacc
import concourse.tile as tile
from concourse import mybir

F32 = mybir.dt.float32
BF16 = mybir.dt.bfloat16
AF = mybir.ActivationFunctionType
OP = mybir.AluOpType

# Model constants (hardcoded per contest contract)
V, E, NH, HS, FF, NB, L = 16, 512, 8, 64, 2048, 6, 512
B, T = 16, 512
TEMP = 1.0 * 0.8
EPS = 1e-5
SCALE = HS ** -0.5  # 0.125

NCORES = 8
SEQ = 2              # sequences per core
NTOK = SEQ * T       # 1024 tokens per core
C = E // 128         # 4 E-chunks
CF = FF // 128       # 16 FF-chunks
HP = NH // 2         # 4 head-pairs
NJ = T // 128        # 4 tk chunks per sequence

_PROGRAM_CACHE = {}


def _emit_ln_tt(nc, pools, h_t, ones_t, eps2_t, g_ap, b_ap, trivial, tt, xn):
    """One token-tile of LayerNorm from bf16 h_t into caller-alloc'd xn.

    Chain is kept DVE-local with a single ACT hop (Sqrt):
      mu = s1/E; var = s2/E - mu^2; sig = sqrt(var + eps); r = 1/sig;
      xn = (h - mu)*r.
    Returns (r_bf, mu_bf) for LN-corrected consumers.
    """
    stats, stats_bf = pools["stats"], pools["stats_bf"]
    ps1 = pools["ps1"]
    sq = pools["sq"]
    sl = slice(tt * 512, tt * 512 + 512)
    s1 = ps1.tile([128, 512], F32, tag="ps1")
    s2 = ps1.tile([128, 512], F32, tag="ps1")
    for c in range(C):
        nc.vector.tensor_tensor(sq[:, c, sl], h_t[:, c, sl], h_t[:, c, sl],
                                OP.mult)
        nc.tensor.matmul(s1[:], ones_t[:], h_t[:, c, sl],
                         start=(c == 0), stop=(c == C - 1))
        nc.tensor.matmul(s2[:], ones_t[:], sq[:, c, sl],
                         start=(c == 0), stop=(c == C - 1))
    mu = stats.tile([128, 512], F32, tag="stats")
    nc.vector.tensor_scalar(out=mu[:], in0=s1[:],
                            scalar1=1.0 / float(E), scalar2=None, op0=OP.mult)
    msq = stats.tile([128, 512], F32, tag="stats")
    nc.vector.tensor_tensor(msq[:], mu[:], mu[:], OP.mult)
    var = stats.tile([128, 512], F32, tag="stats")
    nc.vector.scalar_tensor_tensor(out=var[:], in0=s2[:],
                                   scalar=1.0 / float(E), in1=msq[:],
                                   op0=OP.mult, op1=OP.subtract)
    std = stats.tile([128, 512], F32, tag="stats")
    nc.scalar.activation(std[:], var[:], AF.Sqrt, bias=eps2_t[:])
    rc = stats.tile([128, 512], F32, tag="stats")
    nc.vector.reciprocal_approx_fast(out=rc[:], in_=std[:])
    r_bf = stats_bf.tile([128, 512], BF16, tag="r_bf")
    nc.vector.tensor_scalar(out=r_bf[:], in0=rc[:], scalar1=1.0,
                            scalar2=None, op0=OP.mult)
    mu_bf = stats_bf.tile([128, 512], BF16, tag="mu_bf")
    nc.vector.tensor_scalar(out=mu_bf[:], in0=mu[:], scalar1=1.0,
                            scalar2=None, op0=OP.mult)
    if xn is not None:
        for c in range(C):
            nc.vector.tensor_tensor(xn[:, c, sl], h_t[:, c, sl], mu_bf[:],
                                    OP.subtract)
            nc.vector.tensor_tensor(xn[:, c, sl], xn[:, c, sl], r_bf[:],
                                    OP.mult)
            if not trivial:
                nc.vector.tensor_scalar(out=xn[:, c, sl], in0=xn[:, c, sl],
                                        scalar1=g_ap[:, c:c + 1],
                                        scalar2=b_ap[:, c:c + 1],
                                        op0=OP.mult, op1=OP.add)
    return r_bf, mu_bf


def _alloc_xn(pools):
    return pools["scr"].tile([128, C, NTOK], BF16, tag="scratch", name="xnt")


def _emit_ln(nc, pools, h_t, ones_t, eps2_t, g_ap, b_ap, trivial):
    xn = _alloc_xn(pools)
    rz = []
    for tt in range(2):
        rz.append(_emit_ln_tt(nc, pools, h_t, ones_t, eps2_t, g_ap, b_ap,
                              trivial, tt, xn))
    return xn, rz


def build_program(ln_trivial, bias_zero, nb_run=NB, ln_general_params=True):
    """Build the Bass program.

    ln_trivial: list of NB*2+1 bools (ln1/ln2 per layer then lnf); when True
    the g/b application op is skipped.  bias_zero: (pb, fb1, fb2) all-zero
    flags enabling merged residual/relu fast paths."""
    import os
    corr_en = os.environ.get("KERNEL_CORR", "1") == "1"
    pbz, fb1z, fb2z = bias_zero
    nc = bacc.Bacc(None, target_bir_lowering=False)

    h0_d = nc.dram_tensor("h0", [128, C * NTOK], BF16, kind="ExternalInput")
    qk0_d = nc.dram_tensor("qk0", [128, HP * 2 * NTOK], BF16,
                           kind="ExternalInput")
    v0_d = nc.dram_tensor("v0", [128, SEQ * NJ * 512], BF16,
                          kind="ExternalInput")
    wq_d = nc.dram_tensor("wq", [NB, 128, C * 512], BF16, kind="ExternalInput")
    wk_d = nc.dram_tensor("wk", [NB, 128, C * 512], BF16, kind="ExternalInput")
    wv_d = nc.dram_tensor("wv", [NB, 128, C * 512], BF16, kind="ExternalInput")
    pw_d = nc.dram_tensor("pw", [NB, 128, C * 512], BF16, kind="ExternalInput")
    f1_d = nc.dram_tensor("f1", [NB, 128, C * FF], BF16, kind="ExternalInput")
    f2_d = nc.dram_tensor("f2", [NB, 128, CF * 512], BF16, kind="ExternalInput")
    pb_d = nc.dram_tensor("pb", [128, NB * C], F32, kind="ExternalInput")
    fb1_d = nc.dram_tensor("fb1", [128, NB * CF], F32, kind="ExternalInput")
    fb2_d = nc.dram_tensor("fb2", [128, NB * C], F32, kind="ExternalInput")
    ow_d = nc.dram_tensor("ow", [128, C * V], BF16, kind="ExternalInput")
    ob_d = nc.dram_tensor("ob", [V, 1], F32, kind="ExternalInput")
    tri_d = nc.dram_tensor("tri", [128, 128], BF16, kind="ExternalInput")
    # negated column sums for LN-corrected first blocks (row vectors: these
    # are K=1 matmul stationaries accumulating cs (x) z into PSUM)
    csqk_d = nc.dram_tensor("csqk", [128, NB * 2 * 128], BF16,
                            kind="ExternalInput")
    csf1_d = nc.dram_tensor("csf1", [1, NB * 4 * 128], BF16,
                            kind="ExternalInput")
    csow_d = nc.dram_tensor("csow", [1, V], BF16, kind="ExternalInput")
    lng_d = lnb_d = None
    if ln_general_params:
        lng_d = nc.dram_tensor("lng", [128, (2 * NB + 1) * C], F32,
                               kind="ExternalInput")
        lnb_d = nc.dram_tensor("lnb", [128, (2 * NB + 1) * C], F32,
                               kind="ExternalInput")
    out_d = nc.dram_tensor("logits", [V, NTOK], F32, kind="ExternalOutput")

    from contextlib import ExitStack
    with ExitStack() as ctx:
        tc = ctx.enter_context(tile.TileContext(nc))
        consts = ctx.enter_context(tc.tile_pool(name="consts", bufs=1))
        hpool = ctx.enter_context(tc.tile_pool(name="hpool", bufs=1))
        wqkv = ctx.enter_context(tc.tile_pool(name="wqkv", bufs=1))
        wff1 = ctx.enter_context(tc.tile_pool(name="wff1", bufs=1))
        wff2 = ctx.enter_context(tc.tile_pool(name="wff2", bufs=1))
        scr = ctx.enter_context(tc.tile_pool(name="scr", bufs=3))
        sqpool = ctx.enter_context(tc.tile_pool(name="sqp", bufs=1))
        qk = ctx.enter_context(tc.tile_pool(name="qk", bufs=2))
        vt = ctx.enter_context(tc.tile_pool(name="vt", bufs=1))
        pp = ctx.enter_context(tc.tile_pool(name="pp", bufs=3))
        osb = ctx.enter_context(tc.tile_pool(name="osb", bufs=1))
        ffa = ctx.enter_context(tc.tile_pool(name="ffa", bufs=2))
        corr = ctx.enter_context(tc.tile_pool(name="corr", bufs=2))
        stats = ctx.enter_context(tc.tile_pool(name="stats", bufs=5))
        rdp = ctx.enter_context(tc.tile_pool(name="rdp", bufs=2))
        stats_bf = ctx.enter_context(tc.tile_pool(name="stats_bf", bufs=3))
        ps2 = ctx.enter_context(tc.tile_pool(name="ps2", bufs=3, space="PSUM"))
        ps1 = ctx.enter_context(tc.tile_pool(name="ps1", bufs=2, space="PSUM"))

        sq_t = sqpool.tile([128, C, NTOK], BF16)
        pools = {"scr": scr, "sq": sq_t, "stats": stats, "rdp": rdp,
                 "stats_bf": stats_bf, "ps2": ps2, "ps1": ps1}

        ones_t = consts.tile([128, 128], BF16)
        nc.gpsimd.memset(ones_t[:], 1.0)
        eps2_t = consts.tile([128, 1], F32)
        nc.gpsimd.memset(eps2_t[:], EPS)

        # startup-critical DMAs first: layer-0 attention consumes q/k/v
        # immediately; everything else can stream in behind them.
        qk0_pre = {}
        for hp in (0, 1):
            t = qk.tile([128, 2, NTOK], BF16, tag="qk")
            nc.sync.dma_start(
                t[:], qk0_d[:, hp * 2 * NTOK:(hp + 1) * 2 * NTOK]
                .rearrange("p (q t) -> p q t", t=NTOK))
            qk0_pre[hp] = t
        vt0_pre = vt.tile([128, SEQ * NJ, 512], BF16, tag="vt")
        nc.sync.dma_start(vt0_pre[:], v0_d[:].rearrange(
            "p (g m) -> p g m", m=512))
        tri_t = consts.tile([128, 128], BF16)
        nc.sync.dma_start(tri_t[:], tri_d[:])
        h_t = hpool.tile([128, C, NTOK], BF16)
        nc.sync.dma_start(h_t[:], h0_d[:].rearrange(
            "p (c t) -> p c t", t=NTOK))
        pb_t = consts.tile([128, NB * C], F32)
        nc.sync.dma_start(pb_t[:], pb_d[:])
        fb1_t = consts.tile([128, NB * CF], F32)
        nc.sync.dma_start(fb1_t[:], fb1_d[:])
        fb2_t = consts.tile([128, NB * C], F32)
        nc.sync.dma_start(fb2_t[:], fb2_d[:])
        ow_t = consts.tile([128, C, V], BF16)
        nc.sync.dma_start(ow_t[:], ow_d[:].rearrange("p (c v) -> p c v", v=V))
        ob_t = consts.tile([V, 1], F32)
        nc.sync.dma_start(ob_t[:], ob_d[:])
        # rows 0 and 32 hold hp0/hp1 colsums (legal K=1 tile positions)
        csqk_t = consts.tile([128, NB, 2, 128], BF16)
        nc.sync.dma_start(csqk_t[:], csqk_d[:].rearrange(
            "p (l u m) -> p l u m", u=2, m=128))
        csf1_t = consts.tile([1, NB, 4, 128], BF16)
        nc.sync.dma_start(csf1_t[:], csf1_d[:].rearrange(
            "p (l u m) -> p l u m", u=4, m=128))
        csow_t = consts.tile([1, V], BF16)
        nc.sync.dma_start(csow_t[:], csow_d[:])
        lng_t = lnb_t = None
        if ln_general_params:
            lng_t = consts.tile([128, 2 * NB + 1, C], F32)
            nc.sync.dma_start(lng_t[:], lng_d[:].rearrange(
                "p (l c) -> p l c", c=C))
            lnb_t = consts.tile([128, 2 * NB + 1, C], F32)
            nc.sync.dma_start(lnb_t[:], lnb_d[:].rearrange(
                "p (l c) -> p l c", c=C))

        def ln_params(idx):
            if ln_general_params and not ln_trivial[idx]:
                return lng_t[:, idx, :], lnb_t[:, idx, :], False
            return None, None, True

        lg_sb = consts.tile([V, NTOK], F32)
        final_peeled = False
        for i in range(nb_run):
            # ---- load this layer's weights ----
            if i > 0:
                wq_t = wqkv.tile([128, C, 512], BF16, tag="wq")
                nc.sync.dma_start(wq_t[:], wq_d[i].rearrange(
                    "p (c m) -> p c m", m=512))
                wk_t = wqkv.tile([128, C, 512], BF16, tag="wk")
                nc.sync.dma_start(wk_t[:], wk_d[i].rearrange(
                    "p (c m) -> p c m", m=512))
                wv_t = wqkv.tile([128, C, 512], BF16, tag="wv")
                nc.sync.dma_start(wv_t[:], wv_d[i].rearrange(
                    "p (c m) -> p c m", m=512))
            pw_t = wqkv.tile([128, C, 512], BF16, tag="pw")
            nc.sync.dma_start(pw_t[:], pw_d[i].rearrange(
                "p (c m) -> p c m", m=512))
            f1_t = wff1.tile([128, C, FF], BF16, tag="f1")
            nc.sync.dma_start(f1_t[:], f1_d[i].rearrange(
                "p (c m) -> p c m", m=FF))
            f2_t = wff2.tile([128, CF, 512], BF16, tag="f2")
            nc.sync.dma_start(f2_t[:], f2_d[i].rearrange(
                "p (c m) -> p c m", m=512))

            # ---- LN1 output for this layer (layer 0: host-computed;
            #      others peeled into the previous layer's FFN emission) ----
            xn = xn_next if i > 0 else None
            qk_corr = corr_en and i > 0 and ln_trivial[2 * i]

            # ---- corrected Q/K for head-pairs 0,1: matmuls run on h_t so
            #      the PE has work while the LN1 chain computes r/mu; a K=1
            #      matmul accumulates colsum (x) mu, drain multiplies by r:
            #      q = r*((Wq^T h) - mu*colsum(Wq)) = Wq^T xn exactly. ----
            qk_pre = {}
            if qk_corr:
                for hpc in (0, 1):
                    row = 32 * hpc
                    qk_c = qk.tile([128, 2, NTOK], BF16, tag="qk")
                    mslc = slice(hpc * 128, (hpc + 1) * 128)
                    for tt in range(2):
                        r1, mu1 = rz1_next[tt]
                        sl = slice(tt * 512, tt * 512 + 512)
                        qkp = ps2.tile([128, 2, 512], F32, tag="ps2")
                        for c in range(C):
                            nc.tensor.matmul(qkp[:, 0, :], wq_t[:, c, mslc],
                                             h_t[:, c, sl],
                                             start=(c == 0), stop=False)
                            nc.tensor.matmul(qkp[:, 1, :], wk_t[:, c, mslc],
                                             h_t[:, c, sl],
                                             start=(c == 0), stop=False)
                        for u in range(2):
                            nc.tensor.matmul(
                                qkp[:, u, :],
                                csqk_t[row:row + 1, i, u, :],
                                mu1[row:row + 1, :], start=False, stop=True)
                        nc.vector.tensor_tensor(
                            qk_c[:, :, sl], qkp[:],
                            r1[:, None, :].to_broadcast((128, 2, 512)),
                            OP.mult)
                    qk_pre[hpc] = qk_c
            elif i == 0:
                qk_pre = {0: qk0_pre[0], 1: qk0_pre[1]}

            # ---- V, token-major: vt[tk, hd*64+d]; emitted inside the
            #      attention loop (seq s's chunks during hp=s) so the exps
            #      lead the ACT queue and the V matmuls fill the PE ----
            if i == 0:
                vt_t = vt0_pre
            else:
                vt_t = vt.tile([128, SEQ * NJ, 512], BF16, tag="vt")

            def emit_v_pair(jp):
                vp = ps2.tile([128, 2, 512], F32, tag="ps2")
                for u in range(2):
                    jg = jp * 2 + u
                    for c in range(C):
                        nc.tensor.matmul(
                            vp[:, u, :],
                            xn[:, c, jg * 128:(jg + 1) * 128],
                            wv_t[:, c, :],
                            start=(c == 0), stop=(c == C - 1))
                nc.scalar.copy(vt_t[:, jp * 2:jp * 2 + 2, :], vp[:])

            o_t = osb.tile([128, C, NTOK], BF16, tag="o")

            def emit_den_o(s, hp, p_t):
                base = s * T
                den = ps1.tile([128, 512], F32, tag="ps1")
                for j in range(NJ):
                    off = j * 128
                    njw = T - off
                    for h2 in range(2):
                        nc.tensor.matmul(den[h2 * 64:h2 * 64 + 64, off:512],
                                         ones_t[:, 0:64],
                                         p_t[:, h2, j, 0:njw],
                                         start=(j == 0), stop=(j == NJ - 1),
                                         skip_group_check=True)
                rd = rdp.tile([128, 512], F32, tag="rd")
                nc.vector.reciprocal_approx_fast(out=rd[:], in_=den[:])
                op_ps = ps1.tile([128, 512], F32, tag="ps1")
                for j in range(NJ):
                    off = j * 128
                    njw = T - off
                    for h2 in range(2):
                        head = hp * 2 + h2
                        nc.tensor.matmul(
                            op_ps[h2 * 64:h2 * 64 + 64, off:T],
                            vt_t[:, s * NJ + j, head * 64:head * 64 + 64],
                            p_t[:, h2, j, 0:njw],
                            start=(j == 0), stop=(j == NJ - 1),
                            skip_group_check=True)
                nc.vector.tensor_tensor(o_t[:, hp, base:base + T],
                                        op_ps[:, 0:T], rd[:], OP.mult)

            pending = None
            for hp in range(HP):
                msl = slice(hp * 128, (hp + 1) * 128)
                if hp in qk_pre:
                    qk_t = qk_pre[hp]
                else:
                    qk_t = qk.tile([128, 2, NTOK], BF16, tag="qk")
                    if i == 0:
                        nc.sync.dma_start(
                            qk_t[:],
                            qk0_d[:, hp * 2 * NTOK:(hp + 1) * 2 * NTOK]
                            .rearrange("p (q t) -> p q t", t=NTOK))
                    else:
                        for tt in range(2):
                            sl = slice(tt * 512, tt * 512 + 512)
                            qkp = ps2.tile([128, 2, 512], F32, tag="ps2")
                            for c in range(C):
                                nc.tensor.matmul(qkp[:, 0, :],
                                                 wq_t[:, c, msl],
                                                 xn[:, c, sl],
                                                 start=(c == 0),
                                                 stop=(c == C - 1))
                                nc.tensor.matmul(qkp[:, 1, :],
                                                 wk_t[:, c, msl],
                                                 xn[:, c, sl],
                                                 start=(c == 0),
                                                 stop=(c == C - 1))
                            nc.scalar.copy(qk_t[:, :, sl], qkp[:])

                for s in range(SEQ):
                    base = s * T
                    p_t = pp.tile([128, 2, NJ, 512], BF16, tag="p")
                    for j in range(NJ):
                        off = j * 128
                        njw = T - off
                        sT = ps2.tile([128, 2, 512], F32, tag="ps2")
                        for h2 in range(2):
                            dsl = slice(h2 * 64, h2 * 64 + 64)
                            nc.tensor.matmul(
                                sT[:, h2, 0:njw],
                                qk_t[dsl, 1, base + off:base + off + 128],
                                qk_t[dsl, 0, base + off:base + T],
                                start=True, stop=True)
                        nc.scalar.activation(
                            p_t[:, :, j, 0:njw], sT[:, :, 0:njw],
                            AF.Exp, scale=SCALE)
                        nc.vector.tensor_tensor(
                            p_t[:, :, j, 0:128], p_t[:, :, j, 0:128],
                            tri_t[:, None, :].to_broadcast(
                                (128, 2, 128)), OP.mult)
                    if i > 0 and hp < 2 and s == 0:
                        emit_v_pair(hp * 2)
                        emit_v_pair(hp * 2 + 1)
                    if pending is not None:
                        emit_den_o(*pending)
                    pending = (s, hp, p_t)
            emit_den_o(*pending)

            # ---- attention out projection + residual ----
            for tt in range(2):
                sl = slice(tt * 512, tt * 512 + 512)
                for mcp in range(C // 2):
                    pj = ps2.tile([128, 2, 512], F32, tag="ps2")
                    for u in range(2):
                        mc = mcp * 2 + u
                        for c in range(C):
                            nc.tensor.matmul(
                                pj[:, u, :],
                                pw_t[:, c, mc * 128:(mc + 1) * 128],
                                o_t[:, c, sl],
                                start=(c == 0), stop=(c == C - 1))
                    if pbz:
                        nc.vector.tensor_tensor(
                            h_t[:, mcp * 2:mcp * 2 + 2, sl], pj[:],
                            h_t[:, mcp * 2:mcp * 2 + 2, sl], OP.add)
                    else:
                        for u in range(2):
                            mc = mcp * 2 + u
                            nc.vector.scalar_tensor_tensor(
                                out=h_t[:, mc, sl], in0=pj[:, u, :],
                                scalar=pb_t[:, i * C + mc:i * C + mc + 1],
                                in1=h_t[:, mc, sl], op0=OP.add, op1=OP.add)

            # ---- LN2 + FFN (token-tile split) ----
            g_ap, b_ap, triv = ln_params(2 * i + 1)
            ffn_corr = corr_en and triv
            xn2 = _alloc_xn(pools)
            r2_0, z2_0 = _emit_ln_tt(nc, pools, h_t, ones_t, eps2_t,
                                     g_ap, b_ap, triv, 0, xn2)
            # corrected first FFN1 blocks (tt=0): matmuls on h_t fill the PE
            # while the LN2 chain runs; drain applies r/z + colsum correction.
            corr_fa = []
            if ffn_corr:
                for mfp in range(2):
                    fp = ps2.tile([128, 2, 512], F32, tag="ps2")
                    for u in range(2):
                        mf = mfp * 2 + u
                        for c in range(C):
                            nc.tensor.matmul(
                                fp[:, u, :],
                                f1_t[:, c, mf * 128:(mf + 1) * 128],
                                h_t[:, c, 0:512],
                                start=(c == 0), stop=False)
                    for u in range(2):
                        mf = mfp * 2 + u
                        nc.tensor.matmul(fp[:, u, :], csf1_t[:, i, mf, :],
                                         z2_0[0:1, :], start=False, stop=True)
                    wtmp = corr.tile([128, 2, 512], BF16, tag="corr")
                    nc.vector.tensor_tensor(
                        wtmp[:], fp[:],
                        r2_0[:, None, :].to_broadcast((128, 2, 512)), OP.mult)
                    corr_fa.append(wtmp)
            _emit_ln_tt(nc, pools, h_t, ones_t, eps2_t, g_ap, b_ap, triv,
                        1, xn2)

            for tt in range(2):
                sl = slice(tt * 512, tt * 512 + 512)
                fa = ffa.tile([128, CF, 512], BF16, tag="fa")
                for mfp in range(CF // 2):
                    if tt == 0 and ffn_corr and mfp < 2:
                        src = corr_fa[mfp][:]
                    else:
                        fp = ps2.tile([128, 2, 512], F32, tag="ps2")
                        for u in range(2):
                            mf = mfp * 2 + u
                            for c in range(C):
                                nc.tensor.matmul(
                                    fp[:, u, :],
                                    f1_t[:, c, mf * 128:(mf + 1) * 128],
                                    xn2[:, c, sl],
                                    start=(c == 0), stop=(c == C - 1))
                        src = fp[:]
                    if fb1z:
                        nc.scalar.activation(
                            fa[:, mfp * 2:mfp * 2 + 2, :], src, AF.Relu)
                    else:
                        for u in range(2):
                            mf = mfp * 2 + u
                            nc.scalar.activation(
                                fa[:, mf, :], src[:, u, :], AF.Relu,
                                bias=fb1_t[:, i * CF + mf:i * CF + mf + 1])
                for mcp in range(C // 2):
                    f2p = ps2.tile([128, 2, 512], F32, tag="ps2")
                    for u in range(2):
                        for c16 in range(CF):
                            nc.tensor.matmul(
                                f2p[:, u, :],
                                f2_t[:, c16,
                                     (mcp * 2 + u) * 128:
                                     (mcp * 2 + u + 1) * 128],
                                fa[:, c16, :],
                                start=(c16 == 0), stop=(c16 == CF - 1))
                    if fb2z:
                        nc.vector.tensor_tensor(
                            h_t[:, mcp * 2:mcp * 2 + 2, sl], f2p[:],
                            h_t[:, mcp * 2:mcp * 2 + 2, sl], OP.add)
                    else:
                        for u in range(2):
                            mc = mcp * 2 + u
                            nc.vector.scalar_tensor_tensor(
                                out=h_t[:, mc, sl], in0=f2p[:, u, :],
                                scalar=fb2_t[:, i * C + mc:i * C + mc + 1],
                                in1=h_t[:, mc, sl], op0=OP.add, op1=OP.add)
                # peel next layer's LN1(tt) here so its scalar/vector chain
                # hides behind the other token-tile's FFN matmuls; on the
                # last layer peel the final LN + corrected logits instead
                if i + 1 < nb_run:
                    if tt == 0:
                        xn_next = _alloc_xn(pools)
                        rz1_next = []
                    g_ap, b_ap, triv = ln_params(2 * (i + 1))
                    rz1_next.append(_emit_ln_tt(nc, pools, h_t, ones_t,
                                                eps2_t, g_ap, b_ap, triv,
                                                tt, xn_next))
                else:
                    gf, bf_, trivf = (ln_params(2 * NB) if nb_run == NB
                                      else (None, None, True))
                    if trivf and corr_en:
                        rf, muf = _emit_ln_tt(nc, pools, h_t, ones_t, eps2_t,
                                              gf, bf_, trivf, tt, None)
                        lg = ps1.tile([V, 512], F32, tag="ps1")
                        for c in range(C):
                            nc.tensor.matmul(lg[:], ow_t[:, c, :],
                                             h_t[:, c, sl],
                                             start=(c == 0), stop=False)
                        nc.tensor.matmul(lg[:], csow_t[:], muf[0:1, :],
                                         start=False, stop=True)
                        w16 = rdp.tile([V, 512], F32, tag="rd")
                        nc.vector.tensor_tensor(w16[:], lg[:], rf[0:V, :],
                                                OP.mult)
                        nc.vector.tensor_scalar_add(lg_sb[:, sl], w16[:],
                                                    ob_t[:])
                        final_peeled = True

        # ---- final LN + logits (fallback when not peeled above) ----
        g_ap, b_ap, triv = (ln_params(2 * NB) if nb_run == NB
                            else (None, None, True))
        if not final_peeled:
            xnf, _ = _emit_ln(nc, pools, h_t, ones_t, eps2_t, g_ap, b_ap,
                              triv)
            for tt in range(2):
                sl = slice(tt * 512, tt * 512 + 512)
                lg = ps1.tile([V, 512], F32, tag="ps1")
                for c in range(C):
                    nc.tensor.matmul(lg[:], ow_t[:, c, :], xnf[:, c, sl],
                                     start=(c == 0), stop=(c == C - 1))
                nc.vector.tensor_scalar_add(lg_sb[:, sl], lg[:], ob_t[:])
        nc.sync.dma_start(out_d[:], lg_sb[:])

    nc.finalize()
    return nc


def prepare_inputs(inputs):
    """Host-side preprocessing: embedding gather, layer-0 LN1+QKV, weight
    layout + bf16 cast.  Returns (shared_map, per_core_maps, flags)."""
    f32 = np.float32
    bf16 = ml_dtypes.bfloat16
    x = np.asarray(inputs["x"]).astype(np.int64)
    emb = np.asarray(inputs["emb"], dtype=f32)
    pos = np.asarray(inputs["pos"], dtype=f32)

    positions = np.minimum(np.arange(T), L - 1)
    h0 = (emb[x] + pos[positions][None, :, :]).astype(bf16).astype(f32)

    # layer-0 LN1 + Q/K/V on host (fp32, then bf16)
    g1 = np.asarray(inputs["ln1_g"][0], dtype=f32)
    b1 = np.asarray(inputs["ln1_b"][0], dtype=f32)
    mu = h0.mean(-1, keepdims=True)
    var = np.square(h0 - mu).mean(-1, keepdims=True)
    xn0 = ((h0 - mu) / np.sqrt(var + EPS) * g1 + b1).astype(bf16).astype(f32)
    wq0 = np.asarray(inputs["wq"][0], dtype=f32).astype(bf16).astype(f32)
    wk0 = np.asarray(inputs["wk"][0], dtype=f32).astype(bf16).astype(f32)
    wv0 = np.asarray(inputs["wv"][0], dtype=f32).astype(bf16).astype(f32)
    # [B, T, NH*HS] with head-major feature order
    q0 = np.einsum('bte,hed->bthd', xn0, wq0).reshape(B, T, NH * HS)
    k0 = np.einsum('bte,hed->bthd', xn0, wk0).reshape(B, T, NH * HS)
    v0 = np.einsum('bte,hed->bthd', xn0, wv0).reshape(B, T, NH * HS)

    def to_dev_lhst(mat, kchunks, mcols):
        m = np.ascontiguousarray(mat.astype(bf16))
        return m.reshape(kchunks, 128, mcols).transpose(1, 0, 2).reshape(
            128, kchunks * mcols)

    wq = np.asarray(inputs["wq"], dtype=f32)
    wk = np.asarray(inputs["wk"], dtype=f32)
    wv = np.asarray(inputs["wv"], dtype=f32)
    pw = np.asarray(inputs["proj_w"], dtype=f32)
    f1 = np.asarray(inputs["ff_w1"], dtype=f32)
    f2 = np.asarray(inputs["ff_w2"], dtype=f32)

    wq_dev = np.stack([to_dev_lhst(wq[i].transpose(1, 0, 2).reshape(E, NH * HS),
                                   C, 512) for i in range(NB)])
    wk_dev = np.stack([to_dev_lhst(wk[i].transpose(1, 0, 2).reshape(E, NH * HS),
                                   C, 512) for i in range(NB)])
    wv_dev = np.stack([to_dev_lhst(wv[i].transpose(1, 0, 2).reshape(E, NH * HS),
                                   C, 512) for i in range(NB)])
    pw_dev = np.stack([to_dev_lhst(pw[i], C, 512) for i in range(NB)])
    f1_dev = np.stack([to_dev_lhst(f1[i], C, FF) for i in range(NB)])
    f2_dev = np.stack([to_dev_lhst(f2[i], CF, 512) for i in range(NB)])

    def vec_dev(v, chunks):
        return np.ascontiguousarray(v.astype(f32).reshape(chunks, 128).T)

    pb = np.asarray(inputs["proj_b"], dtype=f32)
    fb1 = np.asarray(inputs["ff_b1"], dtype=f32)
    fb2 = np.asarray(inputs["ff_b2"], dtype=f32)
    bias_zero = (bool(np.all(pb == 0.0)), bool(np.all(fb1 == 0.0)),
                 bool(np.all(fb2 == 0.0)))
    pb_dev = np.concatenate([vec_dev(pb[i], C) for i in range(NB)], axis=1)
    fb1_dev = np.concatenate([vec_dev(fb1[i], CF) for i in range(NB)], axis=1)
    fb2_dev = np.concatenate([vec_dev(fb2[i], C) for i in range(NB)], axis=1)
    ow_dev = to_dev_lhst(np.asarray(inputs["out_w"], dtype=f32) / TEMP, C, V)
    ob_dev = (np.asarray(inputs["out_b"], dtype=f32) / TEMP).reshape(V, 1)
    tri_dev = np.triu(np.ones((128, 128), dtype=f32)).astype(bf16)

    # negated column sums (of the bf16-cast weights) for corrected blocks;
    # shaped as [1, M] row vectors used as K=1 matmul stationaries.
    def neg_cs(mat, cols):
        mb = mat.astype(bf16).astype(f32)
        return -mb[:, cols].sum(axis=0)

    csqk_dev = np.zeros((128, NB, 2, 128), f32)
    csf1_dev = np.zeros((1, NB, 4, 128), f32)
    for i in range(NB):
        wq_flat = wq[i].transpose(1, 0, 2).reshape(E, NH * HS)
        wk_flat = wk[i].transpose(1, 0, 2).reshape(E, NH * HS)
        for hpc in (0, 1):
            cols = slice(hpc * 128, (hpc + 1) * 128)
            csqk_dev[32 * hpc, i, 0] = neg_cs(wq_flat, cols)
            csqk_dev[32 * hpc, i, 1] = neg_cs(wk_flat, cols)
        for mf in range(4):
            csf1_dev[0, i, mf] = neg_cs(f1[i], slice(mf * 128, (mf + 1) * 128))
    csqk_dev = csqk_dev.reshape(128, NB * 2 * 128).astype(bf16)
    csf1_dev = csf1_dev.reshape(1, NB * 4 * 128).astype(bf16)
    csow_dev = np.ascontiguousarray(
        neg_cs(np.asarray(inputs["out_w"], dtype=f32) / TEMP,
               slice(0, V)).reshape(1, V).astype(bf16))

    gs, bs, ln_trivial = [], [], []
    for i in range(NB):
        for nm_g, nm_b in (("ln1_g", "ln1_b"), ("ln2_g", "ln2_b")):
            g = np.asarray(inputs[nm_g][i], dtype=f32)
            b = np.asarray(inputs[nm_b][i], dtype=f32)
            gs.append(vec_dev(g, C))
            bs.append(vec_dev(b, C))
            ln_trivial.append(bool(np.all(g == 1.0) and np.all(b == 0.0)))
    g = np.asarray(inputs["lnf_g"], dtype=f32)
    b = np.asarray(inputs["lnf_b"], dtype=f32)
    gs.append(vec_dev(g, C))
    bs.append(vec_dev(b, C))
    ln_trivial.append(bool(np.all(g == 1.0) and np.all(b == 0.0)))
    lng_dev = np.concatenate(gs, axis=1)
    lnb_dev = np.concatenate(bs, axis=1)

    shared = {
        "wq": wq_dev, "wk": wk_dev, "wv": wv_dev, "pw": pw_dev,
        "f1": f1_dev, "f2": f2_dev, "pb": pb_dev, "fb1": fb1_dev,
        "fb2": fb2_dev, "ow": ow_dev, "ob": ob_dev, "tri": tri_dev,
        "lng": lng_dev, "lnb": lnb_dev, "csqk": csqk_dev, "csf1": csf1_dev,
        "csow": csow_dev,
    }

    per_core = []
    for core in range(NCORES):
        csl = slice(SEQ * core, SEQ * core + SEQ)

        def featmaj(a):                      # [SEQ, T, F] -> [128, F/128*NTOK]
            fT = a[csl].transpose(2, 0, 1).reshape(-1, NTOK)   # [F, NTOK]
            ch = fT.shape[0] // 128
            return np.ascontiguousarray(
                fT.reshape(ch, 128, NTOK).transpose(1, 0, 2).reshape(
                    128, ch * NTOK).astype(bf16))

        h0c = featmaj(h0)                       # [128, C*NTOK]
        # qk0: [128, hp, {q,k}, NTOK]; partition = h2*64+d of the pair
        qf = q0[csl].transpose(2, 0, 1).reshape(NH * HS, NTOK)  # [512, NTOK]
        kf = k0[csl].transpose(2, 0, 1).reshape(NH * HS, NTOK)
        qk0c = np.empty((128, HP, 2, NTOK), dtype=f32)
        for hp in range(HP):
            qk0c[:, hp, 0] = qf[hp * 128:(hp + 1) * 128]
            qk0c[:, hp, 1] = kf[hp * 128:(hp + 1) * 128]
        qk0c = np.ascontiguousarray(
            qk0c.reshape(128, HP * 2 * NTOK).astype(bf16))
        # v0: token-major [128, SEQ*NJ, 512]
        vtok = v0[csl].reshape(NTOK, NH * HS)          # [NTOK, 512]
        v0c = np.ascontiguousarray(
            vtok.reshape(SEQ * NJ, 128, NH * HS).transpose(1, 0, 2).reshape(
                128, SEQ * NJ * 512).astype(bf16))
        per_core.append({"h0": h0c, "qk0": qk0c, "v0": v0c})
    return shared, per_core, (tuple(ln_trivial), bias_zero)


def assemble_output(core_logits):
    """core_logits: list of [V, NTOK] fp32 -> [B, T, V]."""
    out = np.empty((B, T, V), np.float32)
    for core in range(NCORES):
        lg = core_logits[core].reshape(V, SEQ, T)
        out[SEQ * core:SEQ * core + SEQ] = lg.transpose(1, 2, 0)
    return out


def get_program(flags):
    ln_trivial, bias_zero = flags
    key = (ln_trivial, bias_zero)
    if key not in _PROGRAM_CACHE:
        _PROGRAM_CACHE[key] = build_program(list(ln_trivial), bias_zero)
    return _PROGRAM_CACHE[key]


def reset_device():
    """Recover a wedged accelerator (axon session reset). Best-effort."""
    try:
        import ctypes
        import jax
        jax.devices()
        lib = ctypes.CDLL('/opt/axon/libaxon_pjrt.so')
        lib.axon_reset.restype = ctypes.c_int64
        lib.axon_reset()
    except Exception:
        pass


def kernel(**inputs):
    from concourse.bass_utils import run_bass_kernel_spmd
    shared, per_core, flags = prepare_inputs(inputs)
    nc = get_program(flags)
    in_maps = [dict(shared, **per_core[c]) for c in range(NCORES)]
    try:
        res = run_bass_kernel_spmd(nc, in_maps, core_ids=list(range(NCORES)))
    except Exception:
        # A previous (profiled) session can leave the device wedged; reset
        # the axon session and retry once.
        reset_device()
        res = run_bass_kernel_spmd(nc, in_maps, core_ids=list(range(NCORES)))
    return assemble_output([res.results[c]["logits"] for c in range(NCORES)])
